# revision 1
# baseline (speedup 1.0000x reference)
"""BERT-base + CRF loss kernel for 8x Trainium2 NeuronCores.

Strategy (hardcoded for B=16, S=256, H=768, NH=12, DFF=3072, L=12, T=9):
  - Data-parallel over batch: core c processes sequences (2c, 2c+1).
  - Host: embedding gather + embedding LayerNorm (tiny), CRF forward
    algorithm on the [B,S,9] emissions (0.00005% of FLOPs, inherently
    sequential), and folding of zero-cost algebra (bv folded into an
    effective output-projection bias).
  - Device (per core): 12 transformer encoder layers + classifier head
    on 512 tokens. Matmuls in bf16 with fp32 PSUM accumulation;
    residual stream + layernorm statistics in fp32.

Layout: activations are kept in natural layout h[tokens, feat] (so
LayerNorm reduces along the free dim) and transposed per-sublayer into
hT[feat, tokens] via PE-transposes to serve as matmul operands.
Attention computes scoresT[j,i] = k.q per (seq, head), exp on ACT, and
the softmax denominator falls out of the ctx matmul via a ones-column
appended to V. The division by the denominator (free-dim broadcast) is
done with a gpsimd partition_broadcast + one DVE multiply that doubles
as the PSUM->SBUF copy.
"""

import sys

if "/opt/trn_rl_repo" not in sys.path:
    sys.path.insert(0, "/opt/trn_rl_repo")

import contextlib

import numpy as np
import ml_dtypes

import concourse.bass as bass
import concourse.tile as tile
from concourse import bacc, mybir
from concourse import bass_utils
from concourse.masks import make_identity

F32 = mybir.dt.float32
BF16 = mybir.dt.bfloat16
AF = mybir.ActivationFunctionType
ALU = mybir.AluOpType

B, S, V, H, NH, DFF, L, T = 16, 256, 30522, 768, 12, 3072, 12, 9
DH = H // NH  # 64
LN_EPS = 1e-12
N_CORES = 8
TOK = (B // N_CORES) * S  # 512 tokens per core
KT = H // 128  # 6 k-tiles
MT = TOK // 128  # 4 m-tiles
FT = DFF // 128  # 24 ff tiles
NSEQ = B // N_CORES  # 2 sequences per core
HLF = H // 2  # 384, n-half for natural-layout outputs
VW = DH + 1  # 65: v columns per head incl. ones column


def _bld_ln(nc, pools, x, h, gb_sb=None, eps=LN_EPS):
    """LayerNorm along free dim for one m-tile. x/h: [128, H] f32."""
    small = pools["small"]
    stats = small.tile([128, 3, 6], F32, tag="ln_stats")
    for g in range(3):
        nc.vector.bn_stats(stats[:, g, :], x[:, g * 256:(g + 1) * 256])
    mv = small.tile([128, 2], F32, tag="ln_mv")
    nc.vector.bn_aggr(mv[:], stats[:])
    std = small.tile([128, 1], F32, tag="ln_std")
    nc.scalar.activation(std[:], mv[:, 1:2], AF.Sqrt, bias=float(eps))
    rstd = small.tile([128, 1], F32, tag="ln_rstd")
    nc.vector.reciprocal_approx_fast(rstd[:], std[:])
    nc.vector.tensor_scalar(h[:], x, mv[:, 0:1], rstd[:], ALU.subtract, ALU.mult)
    if gb_sb is not None:
        g_b, b_b = gb_sb
        nc.vector.tensor_tensor(h[:], h[:], g_b[:], ALU.mult)
        nc.vector.tensor_tensor(h[:], h[:], b_b[:], ALU.add)


def _bld_transpose(nc, pools, h_list, ident):
    """h_list: MT tiles [128, H] f32 natural -> hT [128, KT, TOK] bf16."""
    hT = pools["hT"].tile([128, KT, TOK], BF16, tag="hT")
    pst = pools["ps_attn"]
    for m in range(MT):
        for k in range(KT):
            pt = pst.tile([128, 256], F32, tag="ps_attn")
            nc.tensor.transpose(pt[:, :128], h_list[m][:, k * 128:(k + 1) * 128],
                                ident)
            nc.any.tensor_copy(hT[:, k, m * 128:(m + 1) * 128], pt[:, :128])
    return hT


def _bld_proj_T(nc, pools, w_sb, hT, bias_col, out_tag):
    """Transposed-output projection: per-k output tiles [128, TOK] bf16."""
    outs = []
    for n in range(KT):
        out = pools["flow"].tile([128, TOK], BF16, tag=out_tag)
        ps = pools["ps_proj"].tile([128, TOK], F32, tag="ps_proj")
        for k in range(KT):
            nc.tensor.matmul(
                ps[:], w_sb[:, k, n * 128:(n + 1) * 128], hT[:, k, :],
                start=(k == 0), stop=(k == KT - 1),
            )
        if bias_col is not None:
            nc.scalar.activation(out[:], ps[:], AF.Identity,
                                 bias=bias_col[:, n:n + 1])
        else:
            nc.scalar.copy(out[:], ps[:])
        outs.append(out)
    return outs


def build_bert(n_layers=L, ln_affine=False, out_bias_rows=False, debug_h=False):
    """Build the bass program. Returns nc.

    ln_affine: emit gamma/beta application (needed when ln params are
    not identity). out_bias_rows: emit ones-row matmuls adding bo_eff/b2
    (needed when those are nonzero)."""
    nc = bacc.Bacc("TRN2", target_bir_lowering=False, debug=False,
                   enable_asserts=False, num_devices=N_CORES)

    d = {}
    d["h0"] = nc.dram_tensor("h0", [TOK, H], F32, kind="ExternalInput").ap()
    for nm in ("Wq", "Wk", "Wv", "Wo"):
        d[nm] = nc.dram_tensor(nm, [n_layers, H, H], BF16, kind="ExternalInput").ap()
    d["W1"] = nc.dram_tensor("W1", [n_layers, H, DFF], BF16, kind="ExternalInput").ap()
    d["W2"] = nc.dram_tensor("W2", [n_layers, DFF, H], BF16, kind="ExternalInput").ap()
    for nm in ("bq", "bk"):
        d[nm] = nc.dram_tensor(nm, [n_layers, H], F32, kind="ExternalInput").ap()
    if out_bias_rows:
        for nm in ("bo_eff", "b2"):
            d[nm] = nc.dram_tensor(nm, [n_layers, H], F32, kind="ExternalInput").ap()
    if ln_affine:
        for nm in ("ln1_g", "ln1_b", "ln2_g", "ln2_b"):
            d[nm + "_bf"] = nc.dram_tensor(nm + "_bf", [n_layers, H], BF16,
                                           kind="ExternalInput").ap()
    d["b1"] = nc.dram_tensor("b1", [n_layers, DFF], F32, kind="ExternalInput").ap()
    d["clf_W"] = nc.dram_tensor("clf_W", [H, T], BF16, kind="ExternalInput").ap()
    d["clf_b"] = nc.dram_tensor("clf_b", [T], F32, kind="ExternalInput").ap()
    logitsT = nc.dram_tensor("logitsT", [T, TOK], F32, kind="ExternalOutput").ap()
    if debug_h:
        dbg = nc.dram_tensor("dbg_h", [n_layers, TOK, H], F32,
                             kind="ExternalOutput").ap()

    with tile.TileContext(nc) as tc:
        with contextlib.ExitStack() as ctx:
            pools = {
                # f32 residual-stream m-tiles [128, H] (3KB/partition each)
                "h": ctx.enter_context(tc.tile_pool(name="h", bufs=8)),
                # per-k bf16 flow tiles [128, TOK] (qT/kT/ctxT)
                "flow": ctx.enter_context(tc.tile_pool(name="flow", bufs=KT)),
                "v": ctx.enter_context(tc.tile_pool(name="v", bufs=MT)),
                "hT": ctx.enter_context(tc.tile_pool(name="hT", bufs=2)),
                "p": ctx.enter_context(tc.tile_pool(name="p", bufs=6)),
                "ff": ctx.enter_context(tc.tile_pool(name="ff", bufs=FT)),
                "w": ctx.enter_context(
                    tc.tile_pool(name="w", bufs=3 if ln_affine else 4)),
                "lnb": ctx.enter_context(tc.tile_pool(name="lnb", bufs=1)),
                "wff": ctx.enter_context(tc.tile_pool(name="wff", bufs=1)),
                "bias": ctx.enter_context(
                    tc.tile_pool(name="bias", bufs=1 if ln_affine else 2)),
                "small": ctx.enter_context(tc.tile_pool(name="small", bufs=4)),
                "small2": ctx.enter_context(tc.tile_pool(name="small2", bufs=2)),
                "const": ctx.enter_context(tc.tile_pool(name="const", bufs=1)),
                "ps_proj": ctx.enter_context(
                    tc.tile_pool(name="ps_proj", bufs=3, space="PSUM")),
                "ps_attn": ctx.enter_context(
                    tc.tile_pool(name="ps_attn", bufs=3, space="PSUM")),
                "ps_ctx": ctx.enter_context(
                    tc.tile_pool(name="ps_ctx", bufs=2, space="PSUM")),
            }

            ident = pools["const"].tile([128, 128], F32, tag="ident")
            make_identity(nc, ident[:])
            # const APs used by nc.scalar.activation float-bias conversion
            zero_c = pools["const"].tile([128, 1], F32, tag="zero_c")
            nc.vector.memset(zero_c[:], 0.0)
            nc.const_aps.aps[(F32, 0.0)] = zero_c[:]
            eps_c = pools["const"].tile([128, 1], F32, tag="eps_c")
            nc.vector.memset(eps_c[:], float(LN_EPS))
            nc.const_aps.aps[(F32, float(LN_EPS))] = eps_c[:]
            if out_bias_rows:
                ones_row = pools["const"].tile([1, 128], F32, tag="ones_row")
                nc.vector.memset(ones_row[:], 1.0)

            h_list = []
            for m in range(MT):
                hm = pools["h"].tile([128, H], F32, tag="h")
                nc.sync.dma_start(
                    hm[:], d["h0"][m * 128:(m + 1) * 128, :])
                h_list.append(hm)

            for l in range(n_layers):
                # ---- per-layer weights/bias loads ----
                def _load_wproj(nm):
                    wt = pools["w"].tile([128, KT, H], BF16, tag="wproj",
                                         name=f"{nm}_{l}")
                    src_ap = d[nm][l].rearrange("(o p) n -> p o n", p=128)
                    # per-k-tile chunks: finer deps let the first matmuls of
                    # each accumulation start before the whole tensor lands
                    for kk in range(KT):
                        nc.sync.dma_start(wt[:, kk, :], src_ap[:, kk, :])
                    return wt
                wq = _load_wproj("Wq")
                wk = _load_wproj("Wk")
                wv = _load_wproj("Wv")
                wo = _load_wproj("Wo")
                w1 = pools["wff"].tile([128, KT, DFF], BF16, tag="w1")
                nc.sync.dma_start(w1[:], d["W1"][l].rearrange("(o p) n -> p o n", p=128))
                w2 = pools["wff"].tile([128, FT, H], BF16, tag="w2")
                nc.sync.dma_start(w2[:], d["W2"][l].rearrange("(o p) n -> p o n", p=128))

                bq = pools["bias"].tile([128, KT], F32, tag="bq")
                nc.sync.dma_start(bq[:], d["bq"][l].rearrange("(o p) -> p o", p=128))
                bk = pools["bias"].tile([128, KT], F32, tag="bk")
                nc.sync.dma_start(bk[:], d["bk"][l].rearrange("(o p) -> p o", p=128))
                b1 = pools["bias"].tile([128, FT], F32, tag="b1")
                nc.sync.dma_start(b1[:], d["b1"][l].rearrange("(o p) -> p o", p=128))

                gb1 = gb2 = None
                if ln_affine:
                    def _ln_bcast(nm):
                        bcast = pools["lnb"].tile([128, H], BF16, tag=nm + "_b")
                        nc.sync.dma_start(
                            bcast[:], d[nm + "_bf"][l][None, :].partition_broadcast(128))
                        return bcast
                    gb1 = [_ln_bcast("ln1_g"), _ln_bcast("ln1_b")]
                    gb2 = [_ln_bcast("ln2_g"), _ln_bcast("ln2_b")]
                bo_row = b2_row = None
                if out_bias_rows:
                    bo_row = pools["lnb"].tile([1, H], F32, tag="bo_row")
                    nc.sync.dma_start(bo_row[:], d["bo_eff"][l][None, :])
                    b2_row = pools["lnb"].tile([1, H], F32, tag="b2_row")
                    nc.sync.dma_start(b2_row[:], d["b2"][l][None, :])

                # ---- transpose h -> hT for QKV ----
                hT = _bld_transpose(nc, pools, h_list, ident[:])

                # ---- Q, K projections (transposed out, per-k tiles) ----
                qT = _bld_proj_T(nc, pools, wq, hT, bq, "qT")
                kT = _bld_proj_T(nc, pools, wk, hT, bk, "kT")

                # ---- V projection (natural out, no bias) + ones cols ----
                v_list = []
                for m in range(MT):
                    vm = pools["v"].tile([128, NH * VW], BF16, tag="v")
                    nc.vector.memset(
                        vm.rearrange("p (h w) -> p h w", w=VW)[:, :, DH], 1.0)
                    v_list.append(vm)
                for m in range(MT):
                    for nh in range(2):
                        ps = pools["ps_proj"].tile([128, TOK], F32, tag="ps_proj")
                        psv = ps[:, :HLF]
                        for k in range(KT):
                            nc.tensor.matmul(
                                psv, hT[:, k, m * 128:(m + 1) * 128],
                                wv[:, k, nh * HLF:(nh + 1) * HLF],
                                start=(k == 0), stop=(k == KT - 1),
                            )
                        for hh in range(NH // 2):
                            h_abs = nh * (NH // 2) + hh
                            nc.vector.tensor_copy(
                                v_list[m][:, h_abs * VW:h_abs * VW + DH],
                                psv[:, hh * DH:(hh + 1) * DH])

                # ---- attention: ktile-outer; scores for both seqs first
                # (pairs adjacent -> concurrent PE row-groups), then ctx; the
                # first 3 output-projection psum groups accumulate per-k in a
                # wave interleaved with attention to keep PE dense ----
                ctxT = []
                for ktile in range(KT):
                    ck = pools["flow"].tile([128, TOK], BF16, tag="ctxT",
                                            name=f"ctx_{l}_{ktile}")
                    ctxT.append(ck)
                x_list = [pools["h"].tile([128, H], F32, tag="h", name=f"x_{l}_{m}")
                          for m in range(MT)]
                hb_list = [pools["h"].tile([128, H], F32, tag="h", name=f"hb_{l}_{m}")
                           for m in range(MT)]
                wave = [(0, 0), (0, 1), (1, 0)]  # (m, nh) groups overlapped
                wave_ps = {}
                for g in wave:
                    wave_ps[g] = pools["ps_proj"].tile([128, TOK], F32,
                                                       tag="ps_proj",
                                                       name=f"wps_{l}_{g[0]}_{g[1]}")
                for ktile in range(KT):
                    p_tiles = {}
                    for s in range(NSEQ):
                        for half in range(2):
                            p_sb = pools["p"].tile([128, 2, 256], BF16, tag="p",
                                                   name=f"p_{l}_{ktile}_{s}_{half}")
                            p_tiles[(s, half)] = p_sb
                        for jt in range(2):
                            for half in range(2):
                                r0 = half * 64
                                ps_s = pools["ps_attn"].tile([128, 256], F32,
                                                             tag="ps_attn")
                                nc.tensor.matmul(
                                    ps_s[:],
                                    kT[ktile][r0:r0 + 64,
                                       s * 256 + jt * 128:s * 256 + (jt + 1) * 128],
                                    qT[ktile][r0:r0 + 64, s * 256:(s + 1) * 256],
                                    start=True, stop=True,
                                )
                                # p = exp(score / sqrt(dh)); no max-subtraction
                                # needed: |score/8| is O(1) here and exp is fp32.
                                nc.scalar.activation(
                                    p_tiles[(s, half)][:, jt, :], ps_s[:], AF.Exp,
                                    scale=float(1.0 / np.sqrt(DH)))
                    # ctx matmuls + 3-phase normalization: batching the
                    # copy/recip, broadcast, and multiply stages across the 4
                    # (seq, half) chains keeps each engine's stream dense
                    # instead of head-serial.
                    for s in range(NSEQ):
                        chains = []
                        for half in range(2):
                            hd = 2 * ktile + half
                            ps_c = pools["ps_ctx"].tile([VW, 256], F32,
                                                        tag="ps_ctx")
                            for jt in range(2):
                                nc.tensor.matmul(
                                    ps_c[:],
                                    v_list[s * 2 + jt][:, hd * VW:(hd + 1) * VW],
                                    p_tiles[(s, half)][:, jt, :],
                                    start=(jt == 0), stop=(jt == 1),
                                )
                            # custom-DVE ops misread PSUM operands on HW:
                            # bounce the sums row through SBUF (on ACT) first.
                            r_sb = pools["small2"].tile([1, 256], F32, tag="r")
                            nc.scalar.copy(r_sb[:], ps_c[DH:VW, :])
                            nc.vector.reciprocal_approx_fast(r_sb[:], r_sb[:])
                            chains.append((half, ps_c, r_sb))
                        rbs = []
                        for half, ps_c, r_sb in chains:
                            rb = pools["small2"].tile([64, 256], F32, tag="rb")
                            nc.gpsimd.partition_broadcast(rb[:], r_sb[:])
                            rbs.append(rb)
                        for (half, ps_c, r_sb), rb in zip(chains, rbs):
                            r0 = half * 64
                            nc.vector.tensor_tensor(
                                ctxT[ktile][r0:r0 + 64, s * 256:(s + 1) * 256],
                                ps_c[:DH, :], rb[:], ALU.mult)
                    # wave: k-th accumulation step for the first 3 out-proj groups
                    for (m, nh) in wave:
                        nc.tensor.matmul(
                            wave_ps[(m, nh)][:, :HLF],
                            ctxT[ktile][:, m * 128:(m + 1) * 128],
                            wo[:, ktile, nh * HLF:(nh + 1) * HLF],
                            start=(ktile == 0), stop=(ktile == KT - 1),
                        )

                # ---- output projection (natural out) + residual + LN1 ----
                for m in range(MT):
                    for nh in range(2):
                        if (m, nh) in wave_ps:
                            pso = wave_ps[(m, nh)][:, :HLF]
                        else:
                            ps = pools["ps_proj"].tile([128, TOK], F32,
                                                       tag="ps_proj")
                            pso = ps[:, :HLF]
                            for k in range(KT):
                                nc.tensor.matmul(
                                    pso, ctxT[k][:, m * 128:(m + 1) * 128],
                                    wo[:, k, nh * HLF:(nh + 1) * HLF],
                                    start=(k == 0), stop=(k == KT - 1),
                                )
                        if out_bias_rows:
                            nc.tensor.matmul(
                                pso, ones_row[:, :128],
                                bo_row[:, nh * HLF:(nh + 1) * HLF],
                                start=False, stop=True, skip_group_check=True,
                            )
                        nc.vector.tensor_tensor(
                            x_list[m][:, nh * HLF:(nh + 1) * HLF], pso,
                            h_list[m][:, nh * HLF:(nh + 1) * HLF], ALU.add)
                    _bld_ln(nc, pools, x_list[m][:], hb_list[m][:], gb1)

                # ---- transpose -> hT_b; FFN1 (transposed out + gelu) ----
                hTb = _bld_transpose(nc, pools, hb_list, ident[:])
                ffT = []
                for n in range(FT):
                    fn = pools["ff"].tile([128, TOK], BF16, tag="ffT")
                    ps = pools["ps_proj"].tile([128, TOK], F32, tag="ps_proj")
                    for k in range(KT):
                        nc.tensor.matmul(
                            ps[:], w1[:, k, n * 128:(n + 1) * 128], hTb[:, k, :],
                            start=(k == 0), stop=(k == KT - 1),
                        )
                    nc.scalar.activation(fn[:], ps[:], AF.Gelu,
                                         bias=b1[:, n:n + 1])
                    ffT.append(fn)

                # ---- FFN2 (natural out) + residual + LN2 -> new h ----
                x2_list = [pools["h"].tile([128, H], F32, tag="h", name=f"x2_{l}_{m}")
                           for m in range(MT)]
                h_list = [pools["h"].tile([128, H], F32, tag="h", name=f"h_{l}_{m}")
                          for m in range(MT)]
                for m in range(MT):
                    for nh in range(2):
                        ps = pools["ps_proj"].tile([128, TOK], F32, tag="ps_proj")
                        psf = ps[:, :HLF]
                        for k in range(FT):
                            nc.tensor.matmul(
                                psf, ffT[k][:, m * 128:(m + 1) * 128],
                                w2[:, k, nh * HLF:(nh + 1) * HLF],
                                start=(k == 0), stop=(k == FT - 1),
                            )
                        if out_bias_rows:
                            nc.tensor.matmul(
                                psf, ones_row[:, :128],
                                b2_row[:, nh * HLF:(nh + 1) * HLF],
                                start=False, stop=True, skip_group_check=True,
                            )
                        nc.vector.tensor_tensor(
                            x2_list[m][:, nh * HLF:(nh + 1) * HLF], psf,
                            hb_list[m][:, nh * HLF:(nh + 1) * HLF], ALU.add)
                    _bld_ln(nc, pools, x2_list[m][:], h_list[m][:], gb2)

                if debug_h:
                    for m in range(MT):
                        nc.sync.dma_start(
                            dbg[l][m * 128:(m + 1) * 128, :], h_list[m][:])

            # ---- classifier ----
            hTf = _bld_transpose(nc, pools, h_list, ident[:])
            wc = pools["bias"].tile([128, KT, T], BF16, tag="wc")
            nc.sync.dma_start(wc[:], d["clf_W"].rearrange("(o p) n -> p o n", p=128))
            bc = pools["bias"].tile([T, 1], F32, tag="bc")
            nc.sync.dma_start(bc[:], d["clf_b"][:, None])
            ps = pools["ps_proj"].tile([128, TOK], F32, tag="ps_proj")
            psl = ps[:T, :]
            for k in range(KT):
                nc.tensor.matmul(psl, wc[:, k, :], hTf[:, k, :],
                                 start=(k == 0), stop=(k == KT - 1))
            lg = pools["const"].tile([T, TOK], F32, tag="lg")
            nc.scalar.activation(lg[:], psl, AF.Identity, bias=bc[:])
            nc.sync.dma_start(logitsT[:], lg[:])

    nc.compile()
    return nc


# ---------------------------------------------------------------------------
# Host side
# ---------------------------------------------------------------------------

def _np(x):
    return np.asarray(x)


def _host_embed(x, word_emb, pos_emb, type_emb, g, b):
    h = word_emb[x] + pos_emb[None, :, :] + type_emb[0][None, None, :]
    m = h.mean(-1, keepdims=True, dtype=np.float32)
    v = ((h - m) ** 2).mean(-1, keepdims=True, dtype=np.float32)
    return ((h - m) / np.sqrt(v + LN_EPS) * g + b).astype(np.float32)


def _logsumexp(a, axis):
    mx = np.max(a, axis=axis, keepdims=True)
    return (mx + np.log(np.sum(np.exp(a - mx), axis=axis, keepdims=True))).squeeze(axis)


def _host_crf(logits, target, crf_start, crf_trans, crf_end):
    logits = logits.astype(np.float32)
    mask = target > -1
    tags = np.where(mask, target, 0)
    bidx = np.arange(B)
    emit = np.take_along_axis(logits, tags[..., None], axis=-1)[..., 0]

    num = crf_start[tags[:, 0]] + emit[:, 0]
    trans = crf_trans[tags[:, :-1], tags[:, 1:]]
    num = num + np.sum((trans + emit[:, 1:]) * mask[:, 1:], axis=1)
    last = np.sum(mask.astype(np.int64), axis=1) - 1
    num = num + crf_end[tags[bidx, last]]

    alpha = crf_start[None, :] + logits[:, 0]
    for t in range(1, S):
        nxt = _logsumexp(alpha[:, :, None] + crf_trans[None], axis=1) + logits[:, t]
        alpha = np.where(mask[:, t][:, None], nxt, alpha)
    denom = _logsumexp(alpha + crf_end[None, :], axis=1)
    llh = num - denom
    return np.float32(-(llh.mean()))


def _ensure_ntff_hook():
    """Dev-only: register the axon NTFF profiling hook if the image's
    antenv package lacks axon_hooks (the boot degrades silently then)."""
    try:
        from antenv.axon_hooks import get_axon_ntff_profile_hook  # noqa: F401
        return
    except ImportError:
        pass
    try:
        import types
        import antenv
        if "/root/.axon_site" not in sys.path:
            sys.path.insert(0, "/root/.axon_site")
        from trn_agent_boot.trn_boot import _ntff_profile_via_ctypes
        hook = _ntff_profile_via_ctypes("/opt/axon/libaxon_pjrt.so")
        mod = types.ModuleType("antenv.axon_hooks")
        state = {"hook": hook}
        mod.get_axon_ntff_profile_hook = lambda: state["hook"]
        mod.set_axon_ntff_profile_hook = lambda h: state.update(hook=h)
        sys.modules["antenv.axon_hooks"] = mod
        antenv.axon_hooks = mod
    except Exception as e:  # profiling is best-effort
        print(f"[kernel] NTFF hook registration failed: {e}")


_CACHE = {}


def _get_nc(ln_affine, out_bias_rows):
    key = ("nc", ln_affine, out_bias_rows)
    if key not in _CACHE:
        _CACHE[key] = build_bert(n_layers=L, ln_affine=ln_affine,
                                 out_bias_rows=out_bias_rows)
    return _CACHE[key]


def expected_input_names(nc):
    names = set()
    for alloc in nc.m.functions[0].allocations:
        if isinstance(alloc, mybir.MemoryLocationSet) and alloc.kind == "ExternalInput":
            names.add(alloc.memorylocations[0].name)
    return names


def _prep_weights(inputs):
    bf = ml_dtypes.bfloat16
    w = {}
    w["Wq"] = _np(inputs["Wq"]).astype(bf)
    w["Wk"] = _np(inputs["Wk"]).astype(bf)
    w["Wv"] = _np(inputs["Wv"]).astype(bf)
    w["Wo"] = _np(inputs["Wo"]).astype(bf)
    w["W1"] = _np(inputs["W1"]).astype(bf)
    w["W2"] = _np(inputs["W2"]).astype(bf)
    w["bq"] = _np(inputs["bq"]).astype(np.float32)
    w["bk"] = _np(inputs["bk"]).astype(np.float32)
    bo = _np(inputs["bo"]).astype(np.float32)
    bv = _np(inputs["bv"]).astype(np.float32)
    Wo = _np(inputs["Wo"]).astype(np.float32)
    # (ctx + bv) @ Wo + bo == ctx @ Wo + (bo + bv @ Wo)
    w["bo_eff"] = (bo + np.einsum("lk,lkn->ln", bv, Wo)).astype(np.float32)
    w["b1"] = _np(inputs["b1"]).astype(np.float32)
    w["b2"] = _np(inputs["b2"]).astype(np.float32)
    for nm in ("ln1_g", "ln1_b", "ln2_g", "ln2_b"):
        w[nm] = _np(inputs[nm]).astype(np.float32)
        w[nm + "_bf"] = _np(inputs[nm]).astype(bf)
    w["clf_W"] = _np(inputs["clf_W"]).astype(bf)
    w["clf_b"] = _np(inputs["clf_b"]).astype(np.float32)
    return w


def kernel(**inputs):
    x = _np(inputs["x"]).astype(np.int64)
    target = _np(inputs["target"]).astype(np.int64)
    h0 = _host_embed(
        x,
        _np(inputs["word_emb"]).astype(np.float32),
        _np(inputs["pos_emb"]).astype(np.float32),
        _np(inputs["type_emb"]).astype(np.float32),
        _np(inputs["emb_ln_g"]).astype(np.float32),
        _np(inputs["emb_ln_b"]).astype(np.float32),
    )  # [B, S, H]

    w = _prep_weights(inputs)
    ln_trivial = (
        np.all(w["ln1_g"] == 1) and np.all(w["ln2_g"] == 1)
        and np.all(w["ln1_b"] == 0) and np.all(w["ln2_b"] == 0)
    )
    ob_trivial = bool(np.all(w["bo_eff"] == 0) and np.all(w["b2"] == 0))

    nc = _get_nc(ln_affine=not ln_trivial, out_bias_rows=not ob_trivial)
    expected = expected_input_names(nc)
    in_maps = []
    for c in range(N_CORES):
        im = {k: v for k, v in w.items() if k in expected}
        im["h0"] = np.ascontiguousarray(
            h0[c * NSEQ:(c + 1) * NSEQ].reshape(TOK, H))
        in_maps.append(im)

    import os
    trace_dir = os.environ.get("BERT_KERNEL_TRACE", "")
    kwargs = {}
    if trace_dir:
        _ensure_ntff_hook()
        os.makedirs(trace_dir, exist_ok=True)
        kwargs = dict(trace=True, tmpdir=trace_dir)
    res = None
    last_err = None
    for attempt in range(3):
        try:
            res = bass_utils.run_bass_kernel_spmd(
                nc, in_maps, core_ids=list(range(N_CORES)), **kwargs)
            break
        except Exception as e:  # transient device errors (NRT_EXEC_UNIT_...)
            last_err = e
            import time as _time
            _time.sleep(5)
    if res is None:
        raise last_err
    if trace_dir:
        print(f"[kernel] exec_time_ns: {res.exec_time_ns}")
        _CACHE["last_results"] = res
    logits = np.empty((B, S, T), np.float32)
    for c in range(N_CORES):
        lt = res.results[c]["logitsT"]  # [T, TOK]
        logits[c * NSEQ:(c + 1) * NSEQ] = lt.T.reshape(NSEQ, S, T)

    return _host_crf(
        logits, target,
        _np(inputs["crf_start"]).astype(np.float32),
        _np(inputs["crf_trans"]).astype(np.float32),
        _np(inputs["crf_end"]).astype(np.float32),
    )



# revision 13
# speedup vs baseline: 1.3185x; 1.3185x over previous
"""BERT-base + CRF loss kernel for 8x Trainium2 NeuronCores.

Strategy (hardcoded for B=16, S=256, H=768, NH=12, DFF=3072, L=12, T=9):
  - Data-parallel over batch: core c processes sequences (2c, 2c+1).
  - Host: embedding gather + embedding LayerNorm (tiny), CRF forward
    algorithm on the [B,S,9] emissions (0.00005% of FLOPs, inherently
    sequential), and folding of zero-cost algebra (bv folded into an
    effective output-projection bias).
  - Device (per core): 12 transformer encoder layers + classifier head
    on 512 tokens. Matmuls in bf16 with fp32 PSUM accumulation;
    residual stream + layernorm statistics in fp32.

Layout: activations are kept in natural layout h[tokens, feat] (so
LayerNorm reduces along the free dim) and transposed per-sublayer into
hT[feat, tokens] via PE-transposes to serve as matmul operands.
Attention computes scoresT[j,i] = k.q per (seq, head), exp on ACT, and
the softmax denominator falls out of the ctx matmul via a ones-column
appended to V. The division by the denominator (free-dim broadcast) is
done with a gpsimd partition_broadcast + one DVE multiply that doubles
as the PSUM->SBUF copy.
"""

import sys

if "/opt/trn_rl_repo" not in sys.path:
    sys.path.insert(0, "/opt/trn_rl_repo")

import contextlib

import numpy as np
import ml_dtypes

import concourse.bass as bass
import concourse.tile as tile
from concourse import bacc, mybir
from concourse import bass_utils
from concourse.masks import make_identity

F32 = mybir.dt.float32
BF16 = mybir.dt.bfloat16
F8 = mybir.dt.float8e4
AF = mybir.ActivationFunctionType
ALU = mybir.AluOpType
DR = mybir.MatmulPerfMode.DoubleRow

B, S, V, H, NH, DFF, L, T = 16, 256, 30522, 768, 12, 3072, 12, 9
DH = H // NH  # 64
LN_EPS = 1e-12
N_CORES = 8
TOK = (B // N_CORES) * S  # 512 tokens per core
KT = H // 128  # 6 k-tiles
KT2 = KT // 2  # 3 DoubleRow k-pairs
MT = TOK // 128  # 4 m-tiles
FT = DFF // 128  # 24 ff tiles
FT2 = FT // 2  # 12 DoubleRow ff-pairs
NSEQ = B // N_CORES  # 2 sequences per core
HLF = H // 2  # 384, n-half for natural-layout outputs
VW = DH + 1  # 65: v columns per head incl. ones column
VPAD = 784  # NH*VW (=780) padded so the v m-stride is 16B-aligned for DoubleRow


def _bld_ln(nc, pools, x, h, gb_sb=None, eps=LN_EPS):
    """LayerNorm along free dim for one m-tile. x/h: [128, H] f32."""
    small = pools["small"]
    stats = small.tile([128, 3, 6], F32, tag="ln_stats")
    for g in range(3):
        nc.vector.bn_stats(stats[:, g, :], x[:, g * 256:(g + 1) * 256])
    mv = small.tile([128, 2], F32, tag="ln_mv")
    nc.vector.bn_aggr(mv[:], stats[:])
    std = small.tile([128, 1], F32, tag="ln_std")
    nc.scalar.activation(std[:], mv[:, 1:2], AF.Sqrt, bias=float(eps))
    rstd = small.tile([128, 1], F32, tag="ln_rstd")
    nc.vector.reciprocal_approx_fast(rstd[:], std[:])
    nc.vector.tensor_scalar(h[:], x, mv[:, 0:1], rstd[:], ALU.subtract, ALU.mult)
    if gb_sb is not None:
        g_b, b_b = gb_sb
        nc.vector.tensor_tensor(h[:], h[:], g_b[:], ALU.mult)
        nc.vector.tensor_tensor(h[:], h[:], b_b[:], ALU.add)


def _bld_transpose(nc, pools, h_list, ident):
    """h_list: MT tiles [128, H] f32 natural -> hT [128, KT, TOK] fp8."""
    hT = pools["hT"].tile([128, KT, TOK], F8, tag="hT")
    pst = pools["ps_attn"]
    for m in range(MT):
        for k in range(KT):
            pt = pst.tile([128, 512], F32, tag="ps_attn")
            nc.tensor.transpose(pt[:, :128], h_list[m][:, k * 128:(k + 1) * 128],
                                ident)
            nc.any.tensor_copy(hT[:, k, m * 128:(m + 1) * 128], pt[:, :128])
    return hT


def _bld_proj_T(nc, pools, w_sb, hT, bias_col, out_tag):
    """Transposed-output projection: per-k output tiles [128, TOK] bf16.

    DoubleRow fp8: each matmul consumes a pair of 128-row k-tiles."""
    outs = []
    for n in range(KT):
        out = pools["flow"].tile([128, TOK], BF16, tag=out_tag)
        ps = pools["ps_proj"].tile([128, TOK], F32, tag="ps_proj")
        for k2 in range(KT2):
            nc.tensor.matmul(
                ps[:], w_sb[:, 2 * k2:2 * k2 + 2, n * 128:(n + 1) * 128],
                hT[:, 2 * k2:2 * k2 + 2, :],
                start=(k2 == 0), stop=(k2 == KT2 - 1), perf_mode=DR,
            )
        if bias_col is not None:
            # bias-add on DVE: keeps ACT free for exp/gelu (fewer table loads)
            nc.vector.tensor_scalar(out[:], ps[:], bias_col[:, n:n + 1], None,
                                    ALU.add)
        else:
            nc.scalar.copy(out[:], ps[:])
        outs.append(out)
    return outs


def build_bert(n_layers=L, ln_affine=False, out_bias_rows=False, debug_h=False):
    """Build the bass program. Returns nc.

    ln_affine: emit gamma/beta application (needed when ln params are
    not identity). out_bias_rows: emit ones-row matmuls adding bo_eff/b2
    (needed when those are nonzero)."""
    nc = bacc.Bacc("TRN2", target_bir_lowering=False, debug=False,
                   enable_asserts=False, num_devices=N_CORES)

    d = {}
    d["h0"] = nc.dram_tensor("h0", [TOK, H], F32, kind="ExternalInput").ap()
    for nm in ("Wq", "Wk", "Wv", "Wo"):
        d[nm] = nc.dram_tensor(nm, [n_layers, H, H], F8, kind="ExternalInput").ap()
    d["W1"] = nc.dram_tensor("W1", [n_layers, H, DFF], F8, kind="ExternalInput").ap()
    d["W2"] = nc.dram_tensor("W2", [n_layers, DFF, H], F8, kind="ExternalInput").ap()
    for nm in ("bq", "bk"):
        d[nm] = nc.dram_tensor(nm, [n_layers, H], F32, kind="ExternalInput").ap()
    if out_bias_rows:
        for nm in ("bo_eff", "b2"):
            d[nm] = nc.dram_tensor(nm, [n_layers, H], F32, kind="ExternalInput").ap()
    if ln_affine:
        for nm in ("ln1_g", "ln1_b", "ln2_g", "ln2_b"):
            d[nm + "_bf"] = nc.dram_tensor(nm + "_bf", [n_layers, H], BF16,
                                           kind="ExternalInput").ap()
    d["b1"] = nc.dram_tensor("b1", [n_layers, DFF], F32, kind="ExternalInput").ap()
    d["clf_W"] = nc.dram_tensor("clf_W", [H, T], F8, kind="ExternalInput").ap()
    d["clf_b"] = nc.dram_tensor("clf_b", [T], F32, kind="ExternalInput").ap()
    logitsT = nc.dram_tensor("logitsT", [T, TOK], F32, kind="ExternalOutput").ap()
    if debug_h:
        dbg = nc.dram_tensor("dbg_h", [n_layers, TOK, H], F32,
                             kind="ExternalOutput").ap()

    with tile.TileContext(nc) as tc:
        with contextlib.ExitStack() as ctx:
            pools = {
                # f32 residual-stream m-tiles [128, H] (3KB/partition each)
                "h": ctx.enter_context(tc.tile_pool(name="h", bufs=8)),
                # per-k bf16 flow tiles [128, TOK] (qT/kT)
                "flow": ctx.enter_context(tc.tile_pool(name="flow", bufs=KT)),
                "v": ctx.enter_context(tc.tile_pool(name="v", bufs=2)),
                "hT": ctx.enter_context(tc.tile_pool(name="hT", bufs=2)),
                "p": ctx.enter_context(tc.tile_pool(name="p", bufs=6)),
                "ff": ctx.enter_context(tc.tile_pool(name="ff", bufs=2)),
                "w": ctx.enter_context(tc.tile_pool(name="w", bufs=6)),
                "lnb": ctx.enter_context(tc.tile_pool(name="lnb", bufs=1)),
                "wff": ctx.enter_context(tc.tile_pool(name="wff", bufs=2)),
                "bias": ctx.enter_context(
                    tc.tile_pool(name="bias", bufs=1 if ln_affine else 2)),
                "small": ctx.enter_context(tc.tile_pool(name="small", bufs=4)),
                "small2": ctx.enter_context(tc.tile_pool(name="small2", bufs=2)),
                "const": ctx.enter_context(tc.tile_pool(name="const", bufs=1)),
                "ps_proj": ctx.enter_context(
                    tc.tile_pool(name="ps_proj", bufs=3, space="PSUM")),
                "ps_attn": ctx.enter_context(
                    tc.tile_pool(name="ps_attn", bufs=3, space="PSUM")),
                "ps_ctx": ctx.enter_context(
                    tc.tile_pool(name="ps_ctx", bufs=2, space="PSUM")),
            }

            ident = pools["const"].tile([128, 128], F32, tag="ident")
            make_identity(nc, ident[:])
            # const APs used by nc.scalar.activation float-bias conversion
            zero_c = pools["const"].tile([128, 1], F32, tag="zero_c")
            nc.vector.memset(zero_c[:], 0.0)
            nc.const_aps.aps[(F32, 0.0)] = zero_c[:]
            eps_c = pools["const"].tile([128, 1], F32, tag="eps_c")
            nc.vector.memset(eps_c[:], float(LN_EPS))
            nc.const_aps.aps[(F32, float(LN_EPS))] = eps_c[:]
            if out_bias_rows:
                ones_row = pools["const"].tile([1, 128], F32, tag="ones_row")
                nc.vector.memset(ones_row[:], 1.0)

            h_list = []
            for m in range(MT):
                hm = pools["h"].tile([128, H], F32, tag="h")
                nc.sync.dma_start(
                    hm[:], d["h0"][m * 128:(m + 1) * 128, :])
                h_list.append(hm)

            for l in range(n_layers):
                # ---- per-layer weights/bias loads ----
                def _load_wproj(nm):
                    wt = pools["w"].tile([128, KT, H], F8, tag="wproj",
                                         name=f"{nm}_{l}")
                    src_ap = d[nm][l].rearrange("(o p) n -> p o n", p=128)
                    # per-k-pair chunks: finer deps let the first matmuls of
                    # each accumulation start before the whole tensor lands
                    for kk in range(KT2):
                        nc.sync.dma_start(wt[:, 2 * kk:2 * kk + 2, :],
                                          src_ap[:, 2 * kk:2 * kk + 2, :])
                    return wt
                wq = _load_wproj("Wq")
                wk = _load_wproj("Wk")
                wv = _load_wproj("Wv")
                wo = _load_wproj("Wo")
                w1 = pools["wff"].tile([128, KT, DFF], F8, tag="w1")
                nc.sync.dma_start(w1[:], d["W1"][l].rearrange("(o p) n -> p o n", p=128))
                w2 = pools["wff"].tile([128, FT, H], F8, tag="w2")
                nc.sync.dma_start(w2[:], d["W2"][l].rearrange("(o p) n -> p o n", p=128))

                bq = pools["bias"].tile([128, KT], F32, tag="bq")
                nc.sync.dma_start(bq[:], d["bq"][l].rearrange("(o p) -> p o", p=128))
                bk = pools["bias"].tile([128, KT], F32, tag="bk")
                nc.sync.dma_start(bk[:], d["bk"][l].rearrange("(o p) -> p o", p=128))
                b1 = pools["bias"].tile([128, FT], F32, tag="b1")
                nc.sync.dma_start(b1[:], d["b1"][l].rearrange("(o p) -> p o", p=128))

                gb1 = gb2 = None
                if ln_affine:
                    def _ln_bcast(nm):
                        bcast = pools["lnb"].tile([128, H], BF16, tag=nm + "_b")
                        nc.sync.dma_start(
                            bcast[:], d[nm + "_bf"][l][None, :].partition_broadcast(128))
                        return bcast
                    gb1 = [_ln_bcast("ln1_g"), _ln_bcast("ln1_b")]
                    gb2 = [_ln_bcast("ln2_g"), _ln_bcast("ln2_b")]
                bo_row = b2_row = None
                if out_bias_rows:
                    bo_row = pools["lnb"].tile([1, H], F32, tag="bo_row")
                    nc.sync.dma_start(bo_row[:], d["bo_eff"][l][None, :])
                    b2_row = pools["lnb"].tile([1, H], F32, tag="b2_row")
                    nc.sync.dma_start(b2_row[:], d["b2"][l][None, :])

                # ---- transpose h -> hT for QKV ----
                hT = _bld_transpose(nc, pools, h_list, ident[:])

                # ---- Q, K projections (transposed out, per-k tiles) ----
                qT = _bld_proj_T(nc, pools, wq, hT, bq, "qT")
                kT = _bld_proj_T(nc, pools, wk, hT, bk, "kT")

                # ---- V projection (natural out, no bias) + ones cols ----
                # single fp8 tile [128, MT, VPAD]: m-pairs are DoubleRow
                # contraction pairs for the ctx matmul
                v_sb = pools["v"].tile([128, MT, VPAD], F8, tag="v")
                nc.vector.memset(
                    v_sb[:, :, :NH * VW].rearrange(
                        "p m (h w) -> p m h w", w=VW)[:, :, :, DH], 1.0)
                for m in range(MT):
                    for nh in range(2):
                        ps = pools["ps_proj"].tile([128, TOK], F32, tag="ps_proj")
                        psv = ps[:, :HLF]
                        for k2 in range(KT2):
                            nc.tensor.matmul(
                                psv, hT[:, 2 * k2:2 * k2 + 2, m * 128:(m + 1) * 128],
                                wv[:, 2 * k2:2 * k2 + 2, nh * HLF:(nh + 1) * HLF],
                                start=(k2 == 0), stop=(k2 == KT2 - 1), perf_mode=DR,
                            )
                        for hh in range(NH // 2):
                            h_abs = nh * (NH // 2) + hh
                            nc.vector.tensor_copy(
                                v_sb[:, m, h_abs * VW:h_abs * VW + DH],
                                psv[:, hh * DH:(hh + 1) * DH])

                # ---- attention: ktile-outer; scores for both seqs first
                # (pairs adjacent -> concurrent PE row-groups), then ctx; the
                # first 3 output-projection psum groups accumulate per-k-pair
                # in a wave interleaved with attention to keep PE dense ----
                ctxT = pools["hT"].tile([128, KT, TOK], F8, tag="ctxT",
                                        name=f"ctx_{l}")
                x_list = [pools["h"].tile([128, H], F32, tag="h", name=f"x_{l}_{m}")
                          for m in range(MT)]
                hb_list = [pools["h"].tile([128, H], F32, tag="h", name=f"hb_{l}_{m}")
                           for m in range(MT)]
                wave = [(0, 0), (0, 1), (1, 0)]  # (m, nh) groups overlapped
                wave_ps = {}
                for g in wave:
                    wave_ps[g] = pools["ps_proj"].tile([128, TOK], F32,
                                                       tag="ps_proj",
                                                       name=f"wps_{l}_{g[0]}_{g[1]}")
                for ktile in range(KT):
                    p_tiles = {}
                    for s in range(NSEQ):
                        for half in range(2):
                            r0 = half * 64
                            # both jt score blocks into one PSUM bank so a
                            # single exp covers them (start=True clears the
                            # bank; the second matmul overwrites its region)
                            ps_s = pools["ps_attn"].tile([128, 512], F32,
                                                         tag="ps_attn")
                            for jt in range(2):
                                nc.tensor.matmul(
                                    ps_s[:, jt * 256:(jt + 1) * 256],
                                    kT[ktile][r0:r0 + 64,
                                       s * 256 + jt * 128:s * 256 + (jt + 1) * 128],
                                    qT[ktile][r0:r0 + 64, s * 256:(s + 1) * 256],
                                    start=(jt == 0), stop=(jt == 1),
                                    skip_group_check=True,
                                )
                            # p = exp(score / sqrt(dh)); no max-subtraction
                            # needed: |score/8| is O(1) here and exp is fp32.
                            p_sb = pools["p"].tile([128, 2, 256], F8, tag="p",
                                                   name=f"p_{l}_{ktile}_{s}_{half}")
                            nc.scalar.activation(
                                p_sb.rearrange("p a b -> p (a b)"), ps_s[:],
                                AF.Exp, scale=float(1.0 / np.sqrt(DH)))
                            p_tiles[(s, half)] = p_sb
                    # ctx matmuls + 3-phase normalization: batching the
                    # copy/recip, broadcast, and multiply stages across the 4
                    # (seq, half) chains keeps each engine's stream dense
                    # instead of head-serial.
                    for s in range(NSEQ):
                        chains = []
                        for half in range(2):
                            hd = 2 * ktile + half
                            ps_c = pools["ps_ctx"].tile([VW, 256], F32,
                                                        tag="ps_ctx")
                            nc.tensor.matmul(
                                ps_c[:],
                                v_sb[:, 2 * s:2 * s + 2, hd * VW:hd * VW + VW],
                                p_tiles[(s, half)][:],
                                start=True, stop=True, perf_mode=DR,
                            )
                            # custom-DVE ops misread PSUM operands on HW:
                            # bounce the sums row through SBUF (on ACT) first.
                            r_sb = pools["small2"].tile([1, 256], F32, tag="r")
                            nc.scalar.copy(r_sb[:], ps_c[DH:VW, :])
                            nc.vector.reciprocal_approx_fast(r_sb[:], r_sb[:])
                            chains.append((half, ps_c, r_sb))
                        rbs = []
                        for half, ps_c, r_sb in chains:
                            rb = pools["small2"].tile([64, 256], F32, tag="rb")
                            nc.gpsimd.partition_broadcast(rb[:], r_sb[:])
                            rbs.append(rb)
                        for (half, ps_c, r_sb), rb in zip(chains, rbs):
                            r0 = half * 64
                            nc.vector.tensor_tensor(
                                ctxT[r0:r0 + 64, ktile, s * 256:(s + 1) * 256],
                                ps_c[:DH, :], rb[:], ALU.mult)
                    # wave: k2-th accumulation step for the first 3 out-proj
                    # groups, once each ktile pair of ctxT is complete
                    if ktile % 2 == 1:
                        k2 = ktile // 2
                        for (m, nh) in wave:
                            nc.tensor.matmul(
                                wave_ps[(m, nh)][:, :HLF],
                                ctxT[:, ktile - 1:ktile + 1, m * 128:(m + 1) * 128],
                                wo[:, ktile - 1:ktile + 1, nh * HLF:(nh + 1) * HLF],
                                start=(k2 == 0), stop=(k2 == KT2 - 1), perf_mode=DR,
                            )

                # ---- output projection (natural out) + residual + LN1 ----
                for m in range(MT):
                    for nh in range(2):
                        if (m, nh) in wave_ps:
                            pso = wave_ps[(m, nh)][:, :HLF]
                        else:
                            ps = pools["ps_proj"].tile([128, TOK], F32,
                                                       tag="ps_proj")
                            pso = ps[:, :HLF]
                            for k2 in range(KT2):
                                nc.tensor.matmul(
                                    pso, ctxT[:, 2 * k2:2 * k2 + 2, m * 128:(m + 1) * 128],
                                    wo[:, 2 * k2:2 * k2 + 2, nh * HLF:(nh + 1) * HLF],
                                    start=(k2 == 0), stop=(k2 == KT2 - 1),
                                    perf_mode=DR,
                                )
                        if out_bias_rows:
                            nc.tensor.matmul(
                                pso, ones_row[:, :128],
                                bo_row[:, nh * HLF:(nh + 1) * HLF],
                                start=False, stop=True, skip_group_check=True,
                            )
                        nc.vector.tensor_tensor(
                            x_list[m][:, nh * HLF:(nh + 1) * HLF], pso,
                            h_list[m][:, nh * HLF:(nh + 1) * HLF], ALU.add)
                    _bld_ln(nc, pools, x_list[m][:], hb_list[m][:], gb1)

                # ---- transpose -> hT_b; FFN1 (transposed out + gelu) ----
                hTb = _bld_transpose(nc, pools, hb_list, ident[:])
                ffT = pools["ff"].tile([128, FT, TOK], F8, tag="ffT")
                for n in range(FT):
                    ps = pools["ps_proj"].tile([128, TOK], F32, tag="ps_proj")
                    for k2 in range(KT2):
                        nc.tensor.matmul(
                            ps[:], w1[:, 2 * k2:2 * k2 + 2, n * 128:(n + 1) * 128],
                            hTb[:, 2 * k2:2 * k2 + 2, :],
                            start=(k2 == 0), stop=(k2 == KT2 - 1), perf_mode=DR,
                        )
                    nc.scalar.activation(ffT[:, n, :], ps[:], AF.Gelu,
                                         bias=b1[:, n:n + 1])

                # ---- FFN2 (natural out) + residual + LN2 -> new h ----
                x2_list = [pools["h"].tile([128, H], F32, tag="h", name=f"x2_{l}_{m}")
                           for m in range(MT)]
                h_list = [pools["h"].tile([128, H], F32, tag="h", name=f"h_{l}_{m}")
                          for m in range(MT)]
                for m in range(MT):
                    for nh in range(2):
                        ps = pools["ps_proj"].tile([128, TOK], F32, tag="ps_proj")
                        psf = ps[:, :HLF]
                        for k2 in range(FT2):
                            nc.tensor.matmul(
                                psf, ffT[:, 2 * k2:2 * k2 + 2, m * 128:(m + 1) * 128],
                                w2[:, 2 * k2:2 * k2 + 2, nh * HLF:(nh + 1) * HLF],
                                start=(k2 == 0), stop=(k2 == FT2 - 1), perf_mode=DR,
                            )
                        if out_bias_rows:
                            nc.tensor.matmul(
                                psf, ones_row[:, :128],
                                b2_row[:, nh * HLF:(nh + 1) * HLF],
                                start=False, stop=True, skip_group_check=True,
                            )
                        nc.vector.tensor_tensor(
                            x2_list[m][:, nh * HLF:(nh + 1) * HLF], psf,
                            hb_list[m][:, nh * HLF:(nh + 1) * HLF], ALU.add)
                    _bld_ln(nc, pools, x2_list[m][:], h_list[m][:], gb2)

                if debug_h:
                    for m in range(MT):
                        nc.sync.dma_start(
                            dbg[l][m * 128:(m + 1) * 128, :], h_list[m][:])

            # ---- classifier ----
            hTf = _bld_transpose(nc, pools, h_list, ident[:])
            wc = pools["bias"].tile([128, KT, T], F8, tag="wc")
            nc.sync.dma_start(wc[:], d["clf_W"].rearrange("(o p) n -> p o n", p=128))
            bc = pools["bias"].tile([T, 1], F32, tag="bc")
            nc.sync.dma_start(bc[:], d["clf_b"][:, None])
            ps = pools["ps_proj"].tile([128, TOK], F32, tag="ps_proj")
            psl = ps[:T, :]
            for k in range(KT):
                nc.tensor.matmul(psl, wc[:, k, :], hTf[:, k, :],
                                 start=(k == 0), stop=(k == KT - 1))
            lg = pools["const"].tile([T, TOK], F32, tag="lg")
            nc.scalar.activation(lg[:], psl, AF.Identity, bias=bc[:])
            nc.sync.dma_start(logitsT[:], lg[:])

    nc.compile()
    return nc


# ---------------------------------------------------------------------------
# Host side
# ---------------------------------------------------------------------------

def _np(x):
    return np.asarray(x)


def _host_embed(x, word_emb, pos_emb, type_emb, g, b):
    h = word_emb[x] + pos_emb[None, :, :] + type_emb[0][None, None, :]
    m = h.mean(-1, keepdims=True, dtype=np.float32)
    v = ((h - m) ** 2).mean(-1, keepdims=True, dtype=np.float32)
    return ((h - m) / np.sqrt(v + LN_EPS) * g + b).astype(np.float32)


def _logsumexp(a, axis):
    mx = np.max(a, axis=axis, keepdims=True)
    return (mx + np.log(np.sum(np.exp(a - mx), axis=axis, keepdims=True))).squeeze(axis)


def _host_crf(logits, target, crf_start, crf_trans, crf_end):
    logits = logits.astype(np.float32)
    mask = target > -1
    tags = np.where(mask, target, 0)
    bidx = np.arange(B)
    emit = np.take_along_axis(logits, tags[..., None], axis=-1)[..., 0]

    num = crf_start[tags[:, 0]] + emit[:, 0]
    trans = crf_trans[tags[:, :-1], tags[:, 1:]]
    num = num + np.sum((trans + emit[:, 1:]) * mask[:, 1:], axis=1)
    last = np.sum(mask.astype(np.int64), axis=1) - 1
    num = num + crf_end[tags[bidx, last]]

    alpha = crf_start[None, :] + logits[:, 0]
    for t in range(1, S):
        nxt = _logsumexp(alpha[:, :, None] + crf_trans[None], axis=1) + logits[:, t]
        alpha = np.where(mask[:, t][:, None], nxt, alpha)
    denom = _logsumexp(alpha + crf_end[None, :], axis=1)
    llh = num - denom
    return np.float32(-(llh.mean()))


def _ensure_ntff_hook():
    """Dev-only: register the axon NTFF profiling hook if the image's
    antenv package lacks axon_hooks (the boot degrades silently then)."""
    try:
        from antenv.axon_hooks import get_axon_ntff_profile_hook  # noqa: F401
        return
    except ImportError:
        pass
    try:
        import types
        import antenv
        if "/root/.axon_site" not in sys.path:
            sys.path.insert(0, "/root/.axon_site")
        from trn_agent_boot.trn_boot import _ntff_profile_via_ctypes
        hook = _ntff_profile_via_ctypes("/opt/axon/libaxon_pjrt.so")
        mod = types.ModuleType("antenv.axon_hooks")
        state = {"hook": hook}
        mod.get_axon_ntff_profile_hook = lambda: state["hook"]
        mod.set_axon_ntff_profile_hook = lambda h: state.update(hook=h)
        sys.modules["antenv.axon_hooks"] = mod
        antenv.axon_hooks = mod
    except Exception as e:  # profiling is best-effort
        print(f"[kernel] NTFF hook registration failed: {e}")


_CACHE = {}


def _get_nc(ln_affine, out_bias_rows):
    key = ("nc", ln_affine, out_bias_rows)
    if key not in _CACHE:
        _CACHE[key] = build_bert(n_layers=L, ln_affine=ln_affine,
                                 out_bias_rows=out_bias_rows)
    return _CACHE[key]


def expected_input_names(nc):
    names = set()
    for alloc in nc.m.functions[0].allocations:
        if isinstance(alloc, mybir.MemoryLocationSet) and alloc.kind == "ExternalInput":
            names.add(alloc.memorylocations[0].name)
    return names


def _prep_weights(inputs):
    bf = ml_dtypes.bfloat16
    f8 = ml_dtypes.float8_e4m3  # TRN fp8e4: inf at S.1111.000, max normal 240
    w = {}
    w["Wq"] = _np(inputs["Wq"]).astype(f8)
    w["Wk"] = _np(inputs["Wk"]).astype(f8)
    w["Wv"] = _np(inputs["Wv"]).astype(f8)
    w["Wo"] = _np(inputs["Wo"]).astype(f8)
    w["W1"] = _np(inputs["W1"]).astype(f8)
    w["W2"] = _np(inputs["W2"]).astype(f8)
    w["bq"] = _np(inputs["bq"]).astype(np.float32)
    w["bk"] = _np(inputs["bk"]).astype(np.float32)
    bo = _np(inputs["bo"]).astype(np.float32)
    bv = _np(inputs["bv"]).astype(np.float32)
    Wo = _np(inputs["Wo"]).astype(np.float32)
    # (ctx + bv) @ Wo + bo == ctx @ Wo + (bo + bv @ Wo)
    w["bo_eff"] = (bo + np.einsum("lk,lkn->ln", bv, Wo)).astype(np.float32)
    w["b1"] = _np(inputs["b1"]).astype(np.float32)
    w["b2"] = _np(inputs["b2"]).astype(np.float32)
    for nm in ("ln1_g", "ln1_b", "ln2_g", "ln2_b"):
        w[nm] = _np(inputs[nm]).astype(np.float32)
        w[nm + "_bf"] = _np(inputs[nm]).astype(bf)
    w["clf_W"] = _np(inputs["clf_W"]).astype(f8)
    w["clf_b"] = _np(inputs["clf_b"]).astype(np.float32)
    return w


def kernel(**inputs):
    x = _np(inputs["x"]).astype(np.int64)
    target = _np(inputs["target"]).astype(np.int64)
    h0 = _host_embed(
        x,
        _np(inputs["word_emb"]).astype(np.float32),
        _np(inputs["pos_emb"]).astype(np.float32),
        _np(inputs["type_emb"]).astype(np.float32),
        _np(inputs["emb_ln_g"]).astype(np.float32),
        _np(inputs["emb_ln_b"]).astype(np.float32),
    )  # [B, S, H]

    w = _prep_weights(inputs)
    ln_trivial = (
        np.all(w["ln1_g"] == 1) and np.all(w["ln2_g"] == 1)
        and np.all(w["ln1_b"] == 0) and np.all(w["ln2_b"] == 0)
    )
    ob_trivial = bool(np.all(w["bo_eff"] == 0) and np.all(w["b2"] == 0))

    nc = _get_nc(ln_affine=not ln_trivial, out_bias_rows=not ob_trivial)
    expected = expected_input_names(nc)
    in_maps = []
    for c in range(N_CORES):
        im = {k: v for k, v in w.items() if k in expected}
        im["h0"] = np.ascontiguousarray(
            h0[c * NSEQ:(c + 1) * NSEQ].reshape(TOK, H))
        in_maps.append(im)

    import os
    trace_dir = os.environ.get("BERT_KERNEL_TRACE", "")
    kwargs = {}
    if trace_dir:
        _ensure_ntff_hook()
        os.makedirs(trace_dir, exist_ok=True)
        kwargs = dict(trace=True, tmpdir=trace_dir)
    res = None
    last_err = None
    for attempt in range(3):
        try:
            res = bass_utils.run_bass_kernel_spmd(
                nc, in_maps, core_ids=list(range(N_CORES)), **kwargs)
            break
        except Exception as e:  # transient device errors (NRT_EXEC_UNIT_...)
            last_err = e
            import time as _time
            _time.sleep(5)
    if res is None:
        raise last_err
    if trace_dir:
        print(f"[kernel] exec_time_ns: {res.exec_time_ns}")
        _CACHE["last_results"] = res
    logits = np.empty((B, S, T), np.float32)
    for c in range(N_CORES):
        lt = res.results[c]["logitsT"]  # [T, TOK]
        logits[c * NSEQ:(c + 1) * NSEQ] = lt.T.reshape(NSEQ, S, T)

    return _host_crf(
        logits, target,
        _np(inputs["crf_start"]).astype(np.float32),
        _np(inputs["crf_trans"]).astype(np.float32),
        _np(inputs["crf_end"]).astype(np.float32),
    )



# revision 23
# speedup vs baseline: 1.3794x; 1.0462x over previous
"""BERT-base + CRF loss kernel for 8x Trainium2 NeuronCores.

Strategy (hardcoded for B=16, S=256, H=768, NH=12, DFF=3072, L=12, T=9):
  - Data-parallel over batch: core c processes sequences (2c, 2c+1).
  - Host: embedding gather + embedding LayerNorm (tiny), CRF forward
    algorithm on the [B,S,9] emissions (0.00005% of FLOPs, inherently
    sequential), and folding of zero-cost algebra (bv folded into an
    effective output-projection bias).
  - Device (per core): 12 transformer encoder layers + classifier head
    on 512 tokens. Matmuls in bf16 with fp32 PSUM accumulation;
    residual stream + layernorm statistics in fp32.

Layout: activations are kept in natural layout h[tokens, feat] (so
LayerNorm reduces along the free dim) and transposed per-sublayer into
hT[feat, tokens] via PE-transposes to serve as matmul operands.
Attention computes scoresT[j,i] = k.q per (seq, head), exp on ACT, and
the softmax denominator falls out of the ctx matmul via a ones-column
appended to V. The division by the denominator (free-dim broadcast) is
done with a gpsimd partition_broadcast + one DVE multiply that doubles
as the PSUM->SBUF copy.
"""

import sys

if "/opt/trn_rl_repo" not in sys.path:
    sys.path.insert(0, "/opt/trn_rl_repo")

import contextlib

import numpy as np
import ml_dtypes

import concourse.bass as bass
import concourse.tile as tile
from concourse import bacc, mybir
from concourse import bass_utils
from concourse.masks import make_identity

F32 = mybir.dt.float32
BF16 = mybir.dt.bfloat16
F8 = mybir.dt.float8e4
AF = mybir.ActivationFunctionType
ALU = mybir.AluOpType
DR = mybir.MatmulPerfMode.DoubleRow

B, S, V, H, NH, DFF, L, T = 16, 256, 30522, 768, 12, 3072, 12, 9
DH = H // NH  # 64
LN_EPS = 1e-12
N_CORES = 8
TOK = (B // N_CORES) * S  # 512 tokens per core
KT = H // 128  # 6 k-tiles
KT2 = KT // 2  # 3 DoubleRow k-pairs
MT = TOK // 128  # 4 m-tiles
FT = DFF // 128  # 24 ff tiles
FT2 = FT // 2  # 12 DoubleRow ff-pairs
NSEQ = B // N_CORES  # 2 sequences per core
HLF = H // 2  # 384, n-half for natural-layout outputs
VW = DH + 1  # 65: v columns per head incl. ones column
VPAD = 784  # NH*VW (=780) padded so the v m-stride is 16B-aligned for DoubleRow


def _bld_ln(nc, pools, x, h, gb_sb=None, eps=LN_EPS):
    """LayerNorm along free dim for one m-tile. x: [128, H] f32, h bf16."""
    small = pools["small"]
    stats = small.tile([128, 3, 6], F32, tag="ln_stats")
    for g in range(3):
        nc.vector.bn_stats(stats[:, g, :], x[:, g * 256:(g + 1) * 256])
    mv = small.tile([128, 2], F32, tag="ln_mv")
    nc.vector.bn_aggr(mv[:], stats[:])
    std = small.tile([128, 1], F32, tag="ln_std")
    nc.scalar.activation(std[:], mv[:, 1:2], AF.Sqrt, bias=float(eps))
    rstd = small.tile([128, 1], F32, tag="ln_rstd")
    nc.vector.reciprocal_approx_fast(rstd[:], std[:])
    nc.vector.tensor_scalar(h[:], x, mv[:, 0:1], rstd[:], ALU.subtract, ALU.mult)
    if gb_sb is not None:
        g_b, b_b = gb_sb
        nc.vector.tensor_tensor(h[:], h[:], g_b[:], ALU.mult)
        nc.vector.tensor_tensor(h[:], h[:], b_b[:], ALU.add)


def _bld_transpose(nc, pools, h_list, ident):
    """h_list: MT tiles [128, H] bf16 natural -> hT [128, KT, TOK] fp8.

    Per m-tile, all KT 128x128 transposes pack into one PSUM bank
    (bf16: 6*128 cols = 1.5KB) so a single strided copy drains it."""
    hT = pools["hT"].tile([128, KT, TOK], F8, tag="hT")
    pst = pools["ps_attn"]
    for m in range(MT):
        pt = pst.tile([128, KT * 128], BF16, tag="ps_attn")
        for k in range(KT):
            nc.tensor.matmul(
                pt[:, k * 128:(k + 1) * 128],
                h_list[m][:, k * 128:(k + 1) * 128], ident,
                is_transpose=True, start=(k == 0), stop=(k == KT - 1),
                skip_group_check=True,
            )
        nc.any.tensor_copy(
            hT[:, :, m * 128:(m + 1) * 128],
            pt.rearrange("p (k c) -> p k c", c=128))
    return hT


def _bld_proj_T(nc, pools, w_sb, hT, bias_col, out_tag):
    """Transposed-output projection: per-k output tiles [128, TOK] bf16.

    DoubleRow fp8: each matmul consumes a pair of 128-row k-tiles."""
    outs = []
    for n in range(KT):
        out = pools["flow"].tile([128, TOK], BF16, tag=out_tag)
        ps = pools["ps_proj"].tile([128, TOK], F32, tag="ps_proj")
        for k2 in range(KT2):
            nc.tensor.matmul(
                ps[:], w_sb[:, 2 * k2:2 * k2 + 2, n * 128:(n + 1) * 128],
                hT[:, 2 * k2:2 * k2 + 2, :],
                start=(k2 == 0), stop=(k2 == KT2 - 1), perf_mode=DR,
            )
        if bias_col is not None:
            # bias-add on DVE: keeps ACT free for exp/gelu (fewer table loads)
            nc.vector.tensor_scalar(out[:], ps[:], bias_col[:, n:n + 1], None,
                                    ALU.add)
        else:
            nc.scalar.copy(out[:], ps[:])
        outs.append(out)
    return outs


def build_bert(n_layers=L, ln_affine=False, out_bias_rows=False, debug_h=False):
    """Build the bass program. Returns nc.

    ln_affine: emit gamma/beta application (needed when ln params are
    not identity). out_bias_rows: emit ones-row matmuls adding bo_eff/b2
    (needed when those are nonzero)."""
    nc = bacc.Bacc("TRN2", target_bir_lowering=False, debug=False,
                   enable_asserts=False, num_devices=N_CORES)

    d = {}
    d["h0"] = nc.dram_tensor("h0", [TOK, H], BF16, kind="ExternalInput").ap()
    for nm in ("Wq", "Wk", "Wv", "Wo"):
        d[nm] = nc.dram_tensor(nm, [n_layers, H, H], F8, kind="ExternalInput").ap()
    d["W1"] = nc.dram_tensor("W1", [n_layers, H, DFF], F8, kind="ExternalInput").ap()
    d["W2"] = nc.dram_tensor("W2", [n_layers, DFF, H], F8, kind="ExternalInput").ap()
    for nm in ("bq", "bk"):
        d[nm] = nc.dram_tensor(nm, [n_layers, H], F32, kind="ExternalInput").ap()
    if out_bias_rows:
        for nm in ("bo_eff", "b2"):
            d[nm] = nc.dram_tensor(nm, [n_layers, H], F32, kind="ExternalInput").ap()
    if ln_affine:
        for nm in ("ln1_g", "ln1_b", "ln2_g", "ln2_b"):
            d[nm + "_bf"] = nc.dram_tensor(nm + "_bf", [n_layers, H], BF16,
                                           kind="ExternalInput").ap()
    d["b1"] = nc.dram_tensor("b1", [n_layers, DFF], F32, kind="ExternalInput").ap()
    d["clf_W"] = nc.dram_tensor("clf_W", [H, T], F8, kind="ExternalInput").ap()
    d["clf_b"] = nc.dram_tensor("clf_b", [T], F32, kind="ExternalInput").ap()
    logitsT = nc.dram_tensor("logitsT", [T, TOK], F32, kind="ExternalOutput").ap()
    if debug_h:
        dbg = nc.dram_tensor("dbg_h", [n_layers, TOK, H], F32,
                             kind="ExternalOutput").ap()

    with tile.TileContext(nc) as tc:
        with contextlib.ExitStack() as ctx:
            pools = {
                # f32 residual-stream m-tiles [128, H] (3KB/partition each)
                "h": ctx.enter_context(tc.tile_pool(name="h", bufs=8)),
                # per-k bf16 flow tiles [128, TOK] (qT/kT)
                "flow": ctx.enter_context(tc.tile_pool(name="flow", bufs=KT)),
                "v": ctx.enter_context(tc.tile_pool(name="v", bufs=2)),
                "hT": ctx.enter_context(tc.tile_pool(name="hT", bufs=2)),
                "p": ctx.enter_context(tc.tile_pool(name="p", bufs=6)),
                "ff": ctx.enter_context(tc.tile_pool(name="ff", bufs=2)),
                "w": ctx.enter_context(tc.tile_pool(name="w", bufs=6)),
                "lnb": ctx.enter_context(tc.tile_pool(name="lnb", bufs=1)),
                "wff": ctx.enter_context(tc.tile_pool(name="wff", bufs=2)),
                "bias": ctx.enter_context(
                    tc.tile_pool(name="bias", bufs=1 if ln_affine else 2)),
                "small": ctx.enter_context(tc.tile_pool(name="small", bufs=4)),
                "small2": ctx.enter_context(tc.tile_pool(name="small2", bufs=2)),
                "const": ctx.enter_context(tc.tile_pool(name="const", bufs=1)),
                "ps_proj": ctx.enter_context(
                    tc.tile_pool(name="ps_proj", bufs=3, space="PSUM")),
                "ps_attn": ctx.enter_context(
                    tc.tile_pool(name="ps_attn", bufs=3, space="PSUM")),
                "ps_ctx": ctx.enter_context(
                    tc.tile_pool(name="ps_ctx", bufs=2, space="PSUM")),
            }

            ident = pools["const"].tile([128, 128], BF16, tag="ident")
            make_identity(nc, ident[:])
            # const APs used by nc.scalar.activation float-bias conversion
            zero_c = pools["const"].tile([128, 1], F32, tag="zero_c")
            nc.vector.memset(zero_c[:], 0.0)
            nc.const_aps.aps[(F32, 0.0)] = zero_c[:]
            eps_c = pools["const"].tile([128, 1], F32, tag="eps_c")
            nc.vector.memset(eps_c[:], float(LN_EPS))
            nc.const_aps.aps[(F32, float(LN_EPS))] = eps_c[:]
            if out_bias_rows:
                ones_row = pools["const"].tile([1, 128], F32, tag="ones_row")
                nc.vector.memset(ones_row[:], 1.0)

            h_list = []
            for m in range(MT):
                hm = pools["h"].tile([128, H], BF16, tag="h")
                nc.sync.dma_start(
                    hm[:], d["h0"][m * 128:(m + 1) * 128, :])
                h_list.append(hm)

            for l in range(n_layers):
                # ---- per-layer weights/bias loads ----
                def _load_wproj(nm):
                    wt = pools["w"].tile([128, KT, H], F8, tag="wproj",
                                         name=f"{nm}_{l}")
                    src_ap = d[nm][l].rearrange("(o p) n -> p o n", p=128)
                    # per-k-pair chunks: finer deps let the first matmuls of
                    # each accumulation start before the whole tensor lands
                    for kk in range(KT2):
                        nc.sync.dma_start(wt[:, 2 * kk:2 * kk + 2, :],
                                          src_ap[:, 2 * kk:2 * kk + 2, :])
                    return wt
                wq = _load_wproj("Wq")
                wk = _load_wproj("Wk")
                wv = _load_wproj("Wv")
                wo = _load_wproj("Wo")
                w1 = pools["wff"].tile([128, KT, DFF], F8, tag="w1")
                nc.sync.dma_start(w1[:], d["W1"][l].rearrange("(o p) n -> p o n", p=128))
                w2 = pools["wff"].tile([128, FT, H], F8, tag="w2")
                nc.sync.dma_start(w2[:], d["W2"][l].rearrange("(o p) n -> p o n", p=128))

                bq = pools["bias"].tile([128, KT], F32, tag="bq")
                nc.sync.dma_start(bq[:], d["bq"][l].rearrange("(o p) -> p o", p=128))
                bk = pools["bias"].tile([128, KT], F32, tag="bk")
                nc.sync.dma_start(bk[:], d["bk"][l].rearrange("(o p) -> p o", p=128))
                b1 = pools["bias"].tile([128, FT], F32, tag="b1")
                nc.sync.dma_start(b1[:], d["b1"][l].rearrange("(o p) -> p o", p=128))

                gb1 = gb2 = None
                if ln_affine:
                    def _ln_bcast(nm):
                        bcast = pools["lnb"].tile([128, H], BF16, tag=nm + "_b")
                        nc.sync.dma_start(
                            bcast[:], d[nm + "_bf"][l][None, :].partition_broadcast(128))
                        return bcast
                    gb1 = [_ln_bcast("ln1_g"), _ln_bcast("ln1_b")]
                    gb2 = [_ln_bcast("ln2_g"), _ln_bcast("ln2_b")]
                bo_row = b2_row = None
                if out_bias_rows:
                    bo_row = pools["lnb"].tile([1, H], F32, tag="bo_row")
                    nc.sync.dma_start(bo_row[:], d["bo_eff"][l][None, :])
                    b2_row = pools["lnb"].tile([1, H], F32, tag="b2_row")
                    nc.sync.dma_start(b2_row[:], d["b2"][l][None, :])

                # ---- transpose h -> hT for QKV ----
                hT = _bld_transpose(nc, pools, h_list, ident[:])

                # ---- Q, K projections (transposed out, per-k tiles) ----
                qT = _bld_proj_T(nc, pools, wq, hT, bq, "qT")
                kT = _bld_proj_T(nc, pools, wk, hT, bk, "kT")

                # ---- V projection (natural out, no bias) + ones cols ----
                # single fp8 tile [128, MT, VPAD]: m-pairs are DoubleRow
                # contraction pairs for the ctx matmul
                v_sb = pools["v"].tile([128, MT, VPAD], F8, tag="v")
                nc.vector.memset(
                    v_sb[:, :, :NH * VW].rearrange(
                        "p m (h w) -> p m h w", w=VW)[:, :, :, DH], 1.0)
                for m in range(MT):
                    for nh in range(2):
                        ps = pools["ps_proj"].tile([128, TOK], F32, tag="ps_proj")
                        psv = ps[:, :HLF]
                        for k2 in range(KT2):
                            nc.tensor.matmul(
                                psv, hT[:, 2 * k2:2 * k2 + 2, m * 128:(m + 1) * 128],
                                wv[:, 2 * k2:2 * k2 + 2, nh * HLF:(nh + 1) * HLF],
                                start=(k2 == 0), stop=(k2 == KT2 - 1), perf_mode=DR,
                            )
                        # one strided copy drains all 6 heads of this half
                        nh0 = nh * (NH // 2)
                        nc.vector.tensor_copy(
                            v_sb[:, m, nh0 * VW:nh0 * VW + 6 * VW].rearrange(
                                "p (h w) -> p h w", w=VW)[:, :, :DH],
                            psv.rearrange("p (h w) -> p h w", w=DH))

                # ---- attention: ktile-outer; scores for both seqs first
                # (pairs adjacent -> concurrent PE row-groups), then ctx; the
                # first 3 output-projection psum groups accumulate per-k-pair
                # in a wave interleaved with attention to keep PE dense ----
                ctxT = pools["hT"].tile([128, KT, TOK], F8, tag="ctxT",
                                        name=f"ctx_{l}")
                x_list = [pools["h"].tile([128, H], F32, tag="h", name=f"x_{l}_{m}")
                          for m in range(MT)]
                hb_list = [pools["h"].tile([128, H], BF16, tag="h", name=f"hb_{l}_{m}")
                           for m in range(MT)]
                wave = [(0, 0), (0, 1), (1, 0)]  # (m, nh) groups overlapped
                wave_ps = {}
                for g in wave:
                    wave_ps[g] = pools["ps_proj"].tile([128, TOK], F32,
                                                       tag="ps_proj",
                                                       name=f"wps_{l}_{g[0]}_{g[1]}")
                for ktile in range(KT):
                    p_tiles = {}
                    for s in range(NSEQ):
                        for half in range(2):
                            r0 = half * 64
                            # both jt score blocks into one PSUM bank so a
                            # single exp covers them (start=True clears the
                            # bank; the second matmul overwrites its region)
                            ps_s = pools["ps_attn"].tile([128, 512], F32,
                                                         tag="ps_attn")
                            for jt in range(2):
                                nc.tensor.matmul(
                                    ps_s[:, jt * 256:(jt + 1) * 256],
                                    kT[ktile][r0:r0 + 64,
                                       s * 256 + jt * 128:s * 256 + (jt + 1) * 128],
                                    qT[ktile][r0:r0 + 64, s * 256:(s + 1) * 256],
                                    start=(jt == 0), stop=(jt == 1),
                                    skip_group_check=True,
                                )
                            # p = exp(score / sqrt(dh)); no max-subtraction
                            # needed: |score/8| is O(1) here and exp is fp32.
                            p_sb = pools["p"].tile([128, 2, 256], F8, tag="p",
                                                   name=f"p_{l}_{ktile}_{s}_{half}")
                            nc.scalar.activation(
                                p_sb.rearrange("p a b -> p (a b)"), ps_s[:],
                                AF.Exp, scale=float(1.0 / np.sqrt(DH)))
                            p_tiles[(s, half)] = p_sb
                    # ctx matmuls + 3-phase normalization: batching the
                    # copy/recip, broadcast, and multiply stages across the 4
                    # (seq, half) chains keeps each engine's stream dense
                    # instead of head-serial.
                    for s in range(NSEQ):
                        chains = []
                        for half in range(2):
                            hd = 2 * ktile + half
                            ps_c = pools["ps_ctx"].tile([VW, 256], F32,
                                                        tag="ps_ctx")
                            nc.tensor.matmul(
                                ps_c[:],
                                v_sb[:, 2 * s:2 * s + 2, hd * VW:hd * VW + VW],
                                p_tiles[(s, half)][:],
                                start=True, stop=True, perf_mode=DR,
                            )
                            # custom-DVE ops misread PSUM operands on HW:
                            # bounce the sums row through SBUF (on DVE —
                            # keeps ACT free for the exp stream) first.
                            r_sb = pools["small2"].tile([1, 256], F32, tag="r")
                            nc.vector.tensor_copy(r_sb[:], ps_c[DH:VW, :])
                            nc.vector.reciprocal_approx_fast(r_sb[:], r_sb[:])
                            chains.append((half, ps_c, r_sb))
                        rbs = []
                        for half, ps_c, r_sb in chains:
                            rb = pools["small2"].tile([64, 256], F32, tag="rb")
                            nc.gpsimd.partition_broadcast(rb[:], r_sb[:])
                            rbs.append(rb)
                        for (half, ps_c, r_sb), rb in zip(chains, rbs):
                            r0 = half * 64
                            nc.vector.tensor_tensor(
                                ctxT[r0:r0 + 64, ktile, s * 256:(s + 1) * 256],
                                ps_c[:DH, :], rb[:], ALU.mult)
                    # wave: k2-th accumulation step for the first 3 out-proj
                    # groups, once each ktile pair of ctxT is complete
                    if ktile % 2 == 1:
                        k2 = ktile // 2
                        for (m, nh) in wave:
                            nc.tensor.matmul(
                                wave_ps[(m, nh)][:, :HLF],
                                ctxT[:, ktile - 1:ktile + 1, m * 128:(m + 1) * 128],
                                wo[:, ktile - 1:ktile + 1, nh * HLF:(nh + 1) * HLF],
                                start=(k2 == 0), stop=(k2 == KT2 - 1), perf_mode=DR,
                            )

                # ---- output projection (natural out) + residual + LN1 ----
                for m in range(MT):
                    for nh in range(2):
                        if (m, nh) in wave_ps:
                            pso = wave_ps[(m, nh)][:, :HLF]
                        else:
                            ps = pools["ps_proj"].tile([128, TOK], F32,
                                                       tag="ps_proj")
                            pso = ps[:, :HLF]
                            for k2 in range(KT2):
                                nc.tensor.matmul(
                                    pso, ctxT[:, 2 * k2:2 * k2 + 2, m * 128:(m + 1) * 128],
                                    wo[:, 2 * k2:2 * k2 + 2, nh * HLF:(nh + 1) * HLF],
                                    start=(k2 == 0), stop=(k2 == KT2 - 1),
                                    perf_mode=DR,
                                )
                        if out_bias_rows:
                            nc.tensor.matmul(
                                pso, ones_row[:, :128],
                                bo_row[:, nh * HLF:(nh + 1) * HLF],
                                start=False, stop=True, skip_group_check=True,
                            )
                        nc.vector.tensor_tensor(
                            x_list[m][:, nh * HLF:(nh + 1) * HLF], pso,
                            h_list[m][:, nh * HLF:(nh + 1) * HLF], ALU.add)
                    _bld_ln(nc, pools, x_list[m][:], hb_list[m][:], gb1)

                # ---- transpose -> hT_b; FFN1 (transposed out + gelu) ----
                hTb = _bld_transpose(nc, pools, hb_list, ident[:])
                ffT = pools["ff"].tile([128, FT, TOK], F8, tag="ffT")
                for n in range(FT):
                    ps = pools["ps_proj"].tile([128, TOK], F32, tag="ps_proj")
                    for k2 in range(KT2):
                        nc.tensor.matmul(
                            ps[:], w1[:, 2 * k2:2 * k2 + 2, n * 128:(n + 1) * 128],
                            hTb[:, 2 * k2:2 * k2 + 2, :],
                            start=(k2 == 0), stop=(k2 == KT2 - 1), perf_mode=DR,
                        )
                    nc.scalar.activation(ffT[:, n, :], ps[:], AF.Gelu,
                                         bias=b1[:, n:n + 1])

                # ---- FFN2 (natural out) + residual + LN2 -> new h ----
                x2_list = [pools["h"].tile([128, H], F32, tag="h", name=f"x2_{l}_{m}")
                           for m in range(MT)]
                h_list = [pools["h"].tile([128, H], BF16, tag="h", name=f"h_{l}_{m}")
                          for m in range(MT)]
                for m in range(MT):
                    for nh in range(2):
                        ps = pools["ps_proj"].tile([128, TOK], F32, tag="ps_proj")
                        psf = ps[:, :HLF]
                        for k2 in range(FT2):
                            nc.tensor.matmul(
                                psf, ffT[:, 2 * k2:2 * k2 + 2, m * 128:(m + 1) * 128],
                                w2[:, 2 * k2:2 * k2 + 2, nh * HLF:(nh + 1) * HLF],
                                start=(k2 == 0), stop=(k2 == FT2 - 1), perf_mode=DR,
                            )
                        if out_bias_rows:
                            nc.tensor.matmul(
                                psf, ones_row[:, :128],
                                b2_row[:, nh * HLF:(nh + 1) * HLF],
                                start=False, stop=True, skip_group_check=True,
                            )
                        nc.vector.tensor_tensor(
                            x2_list[m][:, nh * HLF:(nh + 1) * HLF], psf,
                            hb_list[m][:, nh * HLF:(nh + 1) * HLF], ALU.add)
                    _bld_ln(nc, pools, x2_list[m][:], h_list[m][:], gb2)

                if debug_h:
                    for m in range(MT):
                        nc.sync.dma_start(
                            dbg[l][m * 128:(m + 1) * 128, :], h_list[m][:])

            # ---- classifier ----
            hTf = _bld_transpose(nc, pools, h_list, ident[:])
            wc = pools["bias"].tile([128, KT, T], F8, tag="wc")
            nc.sync.dma_start(wc[:], d["clf_W"].rearrange("(o p) n -> p o n", p=128))
            bc = pools["bias"].tile([T, 1], F32, tag="bc")
            nc.sync.dma_start(bc[:], d["clf_b"][:, None])
            ps = pools["ps_proj"].tile([128, TOK], F32, tag="ps_proj")
            psl = ps[:T, :]
            for k in range(KT):
                nc.tensor.matmul(psl, wc[:, k, :], hTf[:, k, :],
                                 start=(k == 0), stop=(k == KT - 1))
            lg = pools["const"].tile([T, TOK], F32, tag="lg")
            nc.scalar.activation(lg[:], psl, AF.Identity, bias=bc[:])
            nc.sync.dma_start(logitsT[:], lg[:])

    nc.compile()
    return nc


# ---------------------------------------------------------------------------
# Host side
# ---------------------------------------------------------------------------

def _np(x):
    return np.asarray(x)


def _host_embed(x, word_emb, pos_emb, type_emb, g, b):
    h = word_emb[x] + pos_emb[None, :, :] + type_emb[0][None, None, :]
    m = h.mean(-1, keepdims=True, dtype=np.float32)
    v = ((h - m) ** 2).mean(-1, keepdims=True, dtype=np.float32)
    return ((h - m) / np.sqrt(v + LN_EPS) * g + b).astype(np.float32)


def _logsumexp(a, axis):
    mx = np.max(a, axis=axis, keepdims=True)
    return (mx + np.log(np.sum(np.exp(a - mx), axis=axis, keepdims=True))).squeeze(axis)


def _host_crf(logits, target, crf_start, crf_trans, crf_end):
    logits = logits.astype(np.float32)
    mask = target > -1
    tags = np.where(mask, target, 0)
    bidx = np.arange(B)
    emit = np.take_along_axis(logits, tags[..., None], axis=-1)[..., 0]

    num = crf_start[tags[:, 0]] + emit[:, 0]
    trans = crf_trans[tags[:, :-1], tags[:, 1:]]
    num = num + np.sum((trans + emit[:, 1:]) * mask[:, 1:], axis=1)
    last = np.sum(mask.astype(np.int64), axis=1) - 1
    num = num + crf_end[tags[bidx, last]]

    alpha = crf_start[None, :] + logits[:, 0]
    for t in range(1, S):
        nxt = _logsumexp(alpha[:, :, None] + crf_trans[None], axis=1) + logits[:, t]
        alpha = np.where(mask[:, t][:, None], nxt, alpha)
    denom = _logsumexp(alpha + crf_end[None, :], axis=1)
    llh = num - denom
    return np.float32(-(llh.mean()))


def _ensure_ntff_hook():
    """Dev-only: register the axon NTFF profiling hook if the image's
    antenv package lacks axon_hooks (the boot degrades silently then)."""
    try:
        from antenv.axon_hooks import get_axon_ntff_profile_hook  # noqa: F401
        return
    except ImportError:
        pass
    try:
        import types
        import antenv
        if "/root/.axon_site" not in sys.path:
            sys.path.insert(0, "/root/.axon_site")
        from trn_agent_boot.trn_boot import _ntff_profile_via_ctypes
        hook = _ntff_profile_via_ctypes("/opt/axon/libaxon_pjrt.so")
        mod = types.ModuleType("antenv.axon_hooks")
        state = {"hook": hook}
        mod.get_axon_ntff_profile_hook = lambda: state["hook"]
        mod.set_axon_ntff_profile_hook = lambda h: state.update(hook=h)
        sys.modules["antenv.axon_hooks"] = mod
        antenv.axon_hooks = mod
    except Exception as e:  # profiling is best-effort
        print(f"[kernel] NTFF hook registration failed: {e}")


_CACHE = {}


def _get_nc(ln_affine, out_bias_rows):
    key = ("nc", ln_affine, out_bias_rows)
    if key not in _CACHE:
        _CACHE[key] = build_bert(n_layers=L, ln_affine=ln_affine,
                                 out_bias_rows=out_bias_rows)
    return _CACHE[key]


def expected_input_names(nc):
    names = set()
    for alloc in nc.m.functions[0].allocations:
        if isinstance(alloc, mybir.MemoryLocationSet) and alloc.kind == "ExternalInput":
            names.add(alloc.memorylocations[0].name)
    return names


def _prep_weights(inputs):
    bf = ml_dtypes.bfloat16
    f8 = ml_dtypes.float8_e4m3  # TRN fp8e4: inf at S.1111.000, max normal 240
    w = {}
    w["Wq"] = _np(inputs["Wq"]).astype(f8)
    w["Wk"] = _np(inputs["Wk"]).astype(f8)
    w["Wv"] = _np(inputs["Wv"]).astype(f8)
    w["Wo"] = _np(inputs["Wo"]).astype(f8)
    w["W1"] = _np(inputs["W1"]).astype(f8)
    w["W2"] = _np(inputs["W2"]).astype(f8)
    w["bq"] = _np(inputs["bq"]).astype(np.float32)
    w["bk"] = _np(inputs["bk"]).astype(np.float32)
    bo = _np(inputs["bo"]).astype(np.float32)
    bv = _np(inputs["bv"]).astype(np.float32)
    Wo = _np(inputs["Wo"]).astype(np.float32)
    # (ctx + bv) @ Wo + bo == ctx @ Wo + (bo + bv @ Wo)
    w["bo_eff"] = (bo + np.einsum("lk,lkn->ln", bv, Wo)).astype(np.float32)
    w["b1"] = _np(inputs["b1"]).astype(np.float32)
    w["b2"] = _np(inputs["b2"]).astype(np.float32)
    for nm in ("ln1_g", "ln1_b", "ln2_g", "ln2_b"):
        w[nm] = _np(inputs[nm]).astype(np.float32)
        w[nm + "_bf"] = _np(inputs[nm]).astype(bf)
    w["clf_W"] = _np(inputs["clf_W"]).astype(f8)
    w["clf_b"] = _np(inputs["clf_b"]).astype(np.float32)
    return w


def kernel(**inputs):
    x = _np(inputs["x"]).astype(np.int64)
    target = _np(inputs["target"]).astype(np.int64)
    h0 = _host_embed(
        x,
        _np(inputs["word_emb"]).astype(np.float32),
        _np(inputs["pos_emb"]).astype(np.float32),
        _np(inputs["type_emb"]).astype(np.float32),
        _np(inputs["emb_ln_g"]).astype(np.float32),
        _np(inputs["emb_ln_b"]).astype(np.float32),
    )  # [B, S, H]

    w = _prep_weights(inputs)
    ln_trivial = (
        np.all(w["ln1_g"] == 1) and np.all(w["ln2_g"] == 1)
        and np.all(w["ln1_b"] == 0) and np.all(w["ln2_b"] == 0)
    )
    ob_trivial = bool(np.all(w["bo_eff"] == 0) and np.all(w["b2"] == 0))

    nc = _get_nc(ln_affine=not ln_trivial, out_bias_rows=not ob_trivial)
    expected = expected_input_names(nc)
    in_maps = []
    for c in range(N_CORES):
        im = {k: v for k, v in w.items() if k in expected}
        im["h0"] = np.ascontiguousarray(
            h0[c * NSEQ:(c + 1) * NSEQ].reshape(TOK, H)).astype(ml_dtypes.bfloat16)
        in_maps.append(im)

    import os
    trace_dir = os.environ.get("BERT_KERNEL_TRACE", "")
    kwargs = {}
    if trace_dir:
        _ensure_ntff_hook()
        os.makedirs(trace_dir, exist_ok=True)
        kwargs = dict(trace=True, tmpdir=trace_dir)
    res = None
    last_err = None
    for attempt in range(3):
        try:
            res = bass_utils.run_bass_kernel_spmd(
                nc, in_maps, core_ids=list(range(N_CORES)), **kwargs)
            break
        except Exception as e:  # transient device errors (NRT_EXEC_UNIT_...)
            last_err = e
            import time as _time
            _time.sleep(5)
    if res is None:
        raise last_err
    if trace_dir:
        print(f"[kernel] exec_time_ns: {res.exec_time_ns}")
        _CACHE["last_results"] = res
    logits = np.empty((B, S, T), np.float32)
    for c in range(N_CORES):
        lt = res.results[c]["logitsT"]  # [T, TOK]
        logits[c * NSEQ:(c + 1) * NSEQ] = lt.T.reshape(NSEQ, S, T)

    return _host_crf(
        logits, target,
        _np(inputs["crf_start"]).astype(np.float32),
        _np(inputs["crf_trans"]).astype(np.float32),
        _np(inputs["crf_end"]).astype(np.float32),
    )



# revision 25
# speedup vs baseline: 1.4554x; 1.0551x over previous
"""BERT-base + CRF loss kernel for 8x Trainium2 NeuronCores.

Strategy (hardcoded for B=16, S=256, H=768, NH=12, DFF=3072, L=12, T=9):
  - Data-parallel over batch: core c processes sequences (2c, 2c+1).
  - Host: embedding gather + embedding LayerNorm (tiny), CRF forward
    algorithm on the [B,S,9] emissions (0.00005% of FLOPs, inherently
    sequential), and folding of zero-cost algebra (bv folded into an
    effective output-projection bias).
  - Device (per core): 12 transformer encoder layers + classifier head
    on 512 tokens. Matmuls in bf16 with fp32 PSUM accumulation;
    residual stream + layernorm statistics in fp32.

Layout: activations are kept in natural layout h[tokens, feat] (so
LayerNorm reduces along the free dim) and transposed per-sublayer into
hT[feat, tokens] via PE-transposes to serve as matmul operands.
Attention computes scoresT[j,i] = k.q per (seq, head), exp on ACT, and
the softmax denominator falls out of the ctx matmul via a ones-column
appended to V. The division by the denominator (free-dim broadcast) is
done with a gpsimd partition_broadcast + one DVE multiply that doubles
as the PSUM->SBUF copy.
"""

import sys

if "/opt/trn_rl_repo" not in sys.path:
    sys.path.insert(0, "/opt/trn_rl_repo")

import contextlib

import numpy as np
import ml_dtypes

import concourse.bass as bass
import concourse.tile as tile
from concourse import bacc, mybir
from concourse import bass_utils
from concourse.masks import make_identity

F32 = mybir.dt.float32
BF16 = mybir.dt.bfloat16
F8 = mybir.dt.float8e4
AF = mybir.ActivationFunctionType
ALU = mybir.AluOpType
DR = mybir.MatmulPerfMode.DoubleRow

B, S, V, H, NH, DFF, L, T = 16, 256, 30522, 768, 12, 3072, 12, 9
DH = H // NH  # 64
LN_EPS = 1e-12
N_CORES = 8
TOK = (B // N_CORES) * S  # 512 tokens per core
KT = H // 128  # 6 k-tiles
KT2 = KT // 2  # 3 DoubleRow k-pairs
MT = TOK // 128  # 4 m-tiles
FT = DFF // 128  # 24 ff tiles
FT2 = FT // 2  # 12 DoubleRow ff-pairs
NSEQ = B // N_CORES  # 2 sequences per core
HLF = H // 2  # 384, n-half for natural-layout outputs
VW = DH + 1  # 65: v columns per head incl. ones column
VPAD = 784  # NH*VW (=780) padded so the v m-stride is 16B-aligned for DoubleRow


def _bld_ln(nc, pools, x, h, gb_sb=None, eps=LN_EPS):
    """LayerNorm along free dim for one m-tile. x: [128, H] f32, h bf16."""
    small = pools["small"]
    stats = small.tile([128, 3, 6], F32, tag="ln_stats")
    for g in range(3):
        nc.vector.bn_stats(stats[:, g, :], x[:, g * 256:(g + 1) * 256])
    mv = small.tile([128, 2], F32, tag="ln_mv")
    nc.vector.bn_aggr(mv[:], stats[:])
    std = small.tile([128, 1], F32, tag="ln_std")
    nc.scalar.activation(std[:], mv[:, 1:2], AF.Sqrt, bias=float(eps))
    rstd = small.tile([128, 1], F32, tag="ln_rstd")
    nc.vector.reciprocal_approx_fast(rstd[:], std[:])
    nc.vector.tensor_scalar(h[:], x, mv[:, 0:1], rstd[:], ALU.subtract, ALU.mult)
    if gb_sb is not None:
        g_b, b_b = gb_sb
        nc.vector.tensor_tensor(h[:], h[:], g_b[:], ALU.mult)
        nc.vector.tensor_tensor(h[:], h[:], b_b[:], ALU.add)


def _bld_transpose_m(nc, pools, hT, h_m, m, ident):
    """One m-tile [128, H] bf16 natural -> hT[:, :, m-block] fp8.

    All KT 128x128 transposes pack into one PSUM bank (bf16: 6*128 cols
    = 1.5KB) so a single strided copy drains it."""
    pt = pools["ps_attn"].tile([128, KT * 128], BF16, tag="ps_attn")
    for k in range(KT):
        nc.tensor.matmul(
            pt[:, k * 128:(k + 1) * 128],
            h_m[:, k * 128:(k + 1) * 128], ident,
            is_transpose=True, start=(k == 0), stop=(k == KT - 1),
            skip_group_check=True,
        )
    nc.any.tensor_copy(
        hT[:, :, m * 128:(m + 1) * 128],
        pt.rearrange("p (k c) -> p k c", c=128))


def _bld_transpose(nc, pools, h_list, ident):
    """h_list: MT tiles [128, H] bf16 natural -> hT [128, KT, TOK] fp8."""
    hT = pools["hT"].tile([128, KT, TOK], F8, tag="hT")
    for m in range(MT):
        _bld_transpose_m(nc, pools, hT, h_list[m][:], m, ident)
    return hT


def _bld_proj_T(nc, pools, w_sb, hT, bias_col, out_tag):
    """Transposed-output projection: per-k output tiles [128, TOK] bf16.

    DoubleRow fp8: each matmul consumes a pair of 128-row k-tiles."""
    outs = []
    for n in range(KT):
        out = pools["flow"].tile([128, TOK], BF16, tag=out_tag)
        ps = pools["ps_proj"].tile([128, TOK], F32, tag="ps_proj")
        for k2 in range(KT2):
            nc.tensor.matmul(
                ps[:], w_sb[:, 2 * k2:2 * k2 + 2, n * 128:(n + 1) * 128],
                hT[:, 2 * k2:2 * k2 + 2, :],
                start=(k2 == 0), stop=(k2 == KT2 - 1), perf_mode=DR,
            )
        if bias_col is not None:
            # bias-add on DVE: keeps ACT free for exp/gelu (fewer table loads)
            nc.vector.tensor_scalar(out[:], ps[:], bias_col[:, n:n + 1], None,
                                    ALU.add)
        else:
            nc.scalar.copy(out[:], ps[:])
        outs.append(out)
    return outs


def build_bert(n_layers=L, ln_affine=False, out_bias_rows=False, debug_h=False):
    """Build the bass program. Returns nc.

    ln_affine: emit gamma/beta application (needed when ln params are
    not identity). out_bias_rows: emit ones-row matmuls adding bo_eff/b2
    (needed when those are nonzero)."""
    nc = bacc.Bacc("TRN2", target_bir_lowering=False, debug=False,
                   enable_asserts=False, num_devices=N_CORES)

    d = {}
    d["h0"] = nc.dram_tensor("h0", [TOK, H], BF16, kind="ExternalInput").ap()
    for nm in ("Wq", "Wk", "Wv", "Wo"):
        d[nm] = nc.dram_tensor(nm, [n_layers, H, H], F8, kind="ExternalInput").ap()
    d["W1"] = nc.dram_tensor("W1", [n_layers, H, DFF], F8, kind="ExternalInput").ap()
    d["W2"] = nc.dram_tensor("W2", [n_layers, DFF, H], F8, kind="ExternalInput").ap()
    for nm in ("bq", "bk"):
        d[nm] = nc.dram_tensor(nm, [n_layers, H], F32, kind="ExternalInput").ap()
    if out_bias_rows:
        for nm in ("bo_eff", "b2"):
            d[nm] = nc.dram_tensor(nm, [n_layers, H], F32, kind="ExternalInput").ap()
    if ln_affine:
        for nm in ("ln1_g", "ln1_b", "ln2_g", "ln2_b"):
            d[nm + "_bf"] = nc.dram_tensor(nm + "_bf", [n_layers, H], BF16,
                                           kind="ExternalInput").ap()
    d["b1"] = nc.dram_tensor("b1", [n_layers, DFF], F32, kind="ExternalInput").ap()
    d["clf_W"] = nc.dram_tensor("clf_W", [H, T], F8, kind="ExternalInput").ap()
    d["clf_b"] = nc.dram_tensor("clf_b", [T], F32, kind="ExternalInput").ap()
    logitsT = nc.dram_tensor("logitsT", [T, TOK], F32, kind="ExternalOutput").ap()
    if debug_h:
        dbg = nc.dram_tensor("dbg_h", [n_layers, TOK, H], F32,
                             kind="ExternalOutput").ap()

    with tile.TileContext(nc) as tc:
        with contextlib.ExitStack() as ctx:
            pools = {
                # f32 residual-stream m-tiles [128, H] (3KB/partition each)
                "h": ctx.enter_context(tc.tile_pool(name="h", bufs=8)),
                # per-k bf16 flow tiles [128, TOK] (qT/kT)
                "flow": ctx.enter_context(tc.tile_pool(name="flow", bufs=KT)),
                "v": ctx.enter_context(tc.tile_pool(name="v", bufs=2)),
                "hT": ctx.enter_context(tc.tile_pool(name="hT", bufs=2)),
                "p": ctx.enter_context(tc.tile_pool(name="p", bufs=6)),
                "ff": ctx.enter_context(tc.tile_pool(name="ff", bufs=2)),
                "w": ctx.enter_context(tc.tile_pool(name="w", bufs=6)),
                "lnb": ctx.enter_context(tc.tile_pool(name="lnb", bufs=1)),
                "wff": ctx.enter_context(tc.tile_pool(name="wff", bufs=2)),
                "bias": ctx.enter_context(
                    tc.tile_pool(name="bias", bufs=1 if ln_affine else 2)),
                "small": ctx.enter_context(tc.tile_pool(name="small", bufs=4)),
                "small2": ctx.enter_context(tc.tile_pool(name="small2", bufs=2)),
                "const": ctx.enter_context(tc.tile_pool(name="const", bufs=1)),
                "ps_proj": ctx.enter_context(
                    tc.tile_pool(name="ps_proj", bufs=3, space="PSUM")),
                "ps_attn": ctx.enter_context(
                    tc.tile_pool(name="ps_attn", bufs=3, space="PSUM")),
                "ps_ctx": ctx.enter_context(
                    tc.tile_pool(name="ps_ctx", bufs=2, space="PSUM")),
            }

            ident = pools["const"].tile([128, 128], BF16, tag="ident")
            make_identity(nc, ident[:])
            # const APs used by nc.scalar.activation float-bias conversion
            zero_c = pools["const"].tile([128, 1], F32, tag="zero_c")
            nc.vector.memset(zero_c[:], 0.0)
            nc.const_aps.aps[(F32, 0.0)] = zero_c[:]
            eps_c = pools["const"].tile([128, 1], F32, tag="eps_c")
            nc.vector.memset(eps_c[:], float(LN_EPS))
            nc.const_aps.aps[(F32, float(LN_EPS))] = eps_c[:]
            if out_bias_rows:
                ones_row = pools["const"].tile([1, 128], F32, tag="ones_row")
                nc.vector.memset(ones_row[:], 1.0)

            h_list = []
            for m in range(MT):
                hm = pools["h"].tile([128, H], BF16, tag="h")
                nc.sync.dma_start(
                    hm[:], d["h0"][m * 128:(m + 1) * 128, :])
                h_list.append(hm)

            for l in range(n_layers):
                # ---- per-layer weights/bias loads ----
                def _load_wproj(nm):
                    wt = pools["w"].tile([128, KT, H], F8, tag="wproj",
                                         name=f"{nm}_{l}")
                    src_ap = d[nm][l].rearrange("(o p) n -> p o n", p=128)
                    # per-k-pair chunks: finer deps let the first matmuls of
                    # each accumulation start before the whole tensor lands
                    for kk in range(KT2):
                        nc.sync.dma_start(wt[:, 2 * kk:2 * kk + 2, :],
                                          src_ap[:, 2 * kk:2 * kk + 2, :])
                    return wt
                wq = _load_wproj("Wq")
                wk = _load_wproj("Wk")
                wv = _load_wproj("Wv")
                wo = _load_wproj("Wo")
                w1 = pools["wff"].tile([128, KT, DFF], F8, tag="w1")
                nc.sync.dma_start(w1[:], d["W1"][l].rearrange("(o p) n -> p o n", p=128))
                w2 = pools["wff"].tile([128, FT, H], F8, tag="w2")
                nc.sync.dma_start(w2[:], d["W2"][l].rearrange("(o p) n -> p o n", p=128))

                bq = pools["bias"].tile([128, KT], F32, tag="bq")
                nc.sync.dma_start(bq[:], d["bq"][l].rearrange("(o p) -> p o", p=128))
                bk = pools["bias"].tile([128, KT], F32, tag="bk")
                nc.sync.dma_start(bk[:], d["bk"][l].rearrange("(o p) -> p o", p=128))
                b1 = pools["bias"].tile([128, FT], F32, tag="b1")
                nc.sync.dma_start(b1[:], d["b1"][l].rearrange("(o p) -> p o", p=128))

                gb1 = gb2 = None
                if ln_affine:
                    def _ln_bcast(nm):
                        bcast = pools["lnb"].tile([128, H], BF16, tag=nm + "_b")
                        nc.sync.dma_start(
                            bcast[:], d[nm + "_bf"][l][None, :].partition_broadcast(128))
                        return bcast
                    gb1 = [_ln_bcast("ln1_g"), _ln_bcast("ln1_b")]
                    gb2 = [_ln_bcast("ln2_g"), _ln_bcast("ln2_b")]
                bo_row = b2_row = None
                if out_bias_rows:
                    bo_row = pools["lnb"].tile([1, H], F32, tag="bo_row")
                    nc.sync.dma_start(bo_row[:], d["bo_eff"][l][None, :])
                    b2_row = pools["lnb"].tile([1, H], F32, tag="b2_row")
                    nc.sync.dma_start(b2_row[:], d["b2"][l][None, :])

                # ---- transpose h -> hT for QKV ----
                hT = _bld_transpose(nc, pools, h_list, ident[:])

                # ---- Q, K projections (transposed out, per-k tiles) ----
                qT = _bld_proj_T(nc, pools, wq, hT, bq, "qT")
                kT = _bld_proj_T(nc, pools, wk, hT, bk, "kT")

                # ---- V projection (natural out, no bias) + ones cols ----
                # single fp8 tile [128, MT, VPAD]: m-pairs are DoubleRow
                # contraction pairs for the ctx matmul. Only the (m, nh=0)
                # groups for seq0's m-tiles are emitted up front; the rest
                # interleave into seq0's softmax-chain window to keep PE fed.
                v_sb = pools["v"].tile([128, MT, VPAD], F8, tag="v")
                nc.vector.memset(
                    v_sb[:, :, :NH * VW].rearrange(
                        "p m (h w) -> p m h w", w=VW)[:, :, :, DH], 1.0)

                def emit_v_group(m, nh):
                    ps = pools["ps_proj"].tile([128, TOK], F32, tag="ps_proj")
                    psv = ps[:, :HLF]
                    for k2 in range(KT2):
                        nc.tensor.matmul(
                            psv, hT[:, 2 * k2:2 * k2 + 2, m * 128:(m + 1) * 128],
                            wv[:, 2 * k2:2 * k2 + 2, nh * HLF:(nh + 1) * HLF],
                            start=(k2 == 0), stop=(k2 == KT2 - 1), perf_mode=DR,
                        )
                    # one strided copy drains all 6 heads of this half
                    nh0 = nh * (NH // 2)
                    nc.vector.tensor_copy(
                        v_sb[:, m, nh0 * VW:nh0 * VW + 6 * VW].rearrange(
                            "p (h w) -> p h w", w=VW)[:, :, :DH],
                        psv.rearrange("p (h w) -> p h w", w=DH))

                emit_v_group(0, 0)
                emit_v_group(1, 0)

                # ---- attention: seq-outer. Chains for seq s run their
                # softmax round-trips (ACT exp -> PE ctx -> DVE/gpsimd
                # normalize) while PE interleaves independent work: remaining
                # V groups during seq0, seq0's output projection + LN1 +
                # transposes during seq1. Per (ktile, seq) both head-halves
                # share one scores bank, one ctx bank, one recip/broadcast. --
                ctxT = {
                    si: pools["hT"].tile([128, KT, 256], F8, tag="ctxT",
                                         name=f"ctx_{l}_{si}")
                    for si in range(NSEQ)
                }
                x_list = [pools["h"].tile([128, H], F32, tag="h", name=f"x_{l}_{m}")
                          for m in range(MT)]
                hb_list = [pools["h"].tile([128, H], BF16, tag="h", name=f"hb_{l}_{m}")
                           for m in range(MT)]
                hTb = pools["hT"].tile([128, KT, TOK], F8, tag="hT",
                                       name=f"hTb_{l}")

                def emit_scores(s, ktile):
                    p_half = []
                    for half in range(2):
                        r0 = half * 64
                        # both jt score blocks into one PSUM bank so a
                        # single exp covers them (start=True clears the
                        # bank; the second matmul overwrites its region)
                        ps_s = pools["ps_attn"].tile([128, 512], F32,
                                                     tag="ps_attn")
                        for jt in range(2):
                            nc.tensor.matmul(
                                ps_s[:, jt * 256:(jt + 1) * 256],
                                kT[ktile][r0:r0 + 64,
                                   s * 256 + jt * 128:s * 256 + (jt + 1) * 128],
                                qT[ktile][r0:r0 + 64, s * 256:(s + 1) * 256],
                                start=(jt == 0), stop=(jt == 1),
                                skip_group_check=True,
                            )
                        # p = exp(score / sqrt(dh)); no max-subtraction
                        # needed: |score/8| is O(1) here and exp is fp32.
                        p_sb = pools["p"].tile([128, 2, 256], F8, tag="p",
                                               name=f"p_{l}_{ktile}_{s}_{half}")
                        nc.scalar.activation(
                            p_sb.rearrange("p a b -> p (a b)"), ps_s[:],
                            AF.Exp, scale=float(1.0 / np.sqrt(DH)))
                        p_half.append(p_sb)
                    return p_half

                def emit_ctx(s, ktile, p_half):
                    # both halves' ctx into one bank: cols 0:256 half0,
                    # 256:512 half1 -> one sums-row copy/recip/broadcast
                    ps_c = pools["ps_ctx"].tile([VW, 512], F32, tag="ps_ctx")
                    for half in range(2):
                        hd = 2 * ktile + half
                        nc.tensor.matmul(
                            ps_c[:, half * 256:(half + 1) * 256],
                            v_sb[:, 2 * s:2 * s + 2, hd * VW:hd * VW + VW],
                            p_half[half][:],
                            start=(half == 0), stop=(half == 1),
                            skip_group_check=True, perf_mode=DR,
                        )
                    # custom-DVE ops misread PSUM operands on HW: bounce the
                    # sums row through SBUF (on ACT; DVE is busier) first.
                    r_sb = pools["small2"].tile([1, 512], F32, tag="r")
                    nc.scalar.copy(r_sb[:], ps_c[DH:VW, :])
                    nc.vector.reciprocal_approx_fast(r_sb[:], r_sb[:])
                    rb = pools["small2"].tile([64, 512], F32, tag="rb")
                    nc.gpsimd.partition_broadcast(rb[:], r_sb[:])
                    for half in range(2):
                        r0 = half * 64
                        nc.vector.tensor_tensor(
                            ctxT[s][r0:r0 + 64, ktile, :],
                            ps_c[:DH, half * 256:(half + 1) * 256],
                            rb[:, half * 256:(half + 1) * 256], ALU.mult)

                def emit_wo_group(m, nh):
                    ps = pools["ps_proj"].tile([128, TOK], F32, tag="ps_proj")
                    pso = ps[:, :HLF]
                    for k2 in range(KT2):
                        nc.tensor.matmul(
                            pso, ctxT[m // 2][:, 2 * k2:2 * k2 + 2,
                                              (m % 2) * 128:(m % 2 + 1) * 128],
                            wo[:, 2 * k2:2 * k2 + 2, nh * HLF:(nh + 1) * HLF],
                            start=(k2 == 0), stop=(k2 == KT2 - 1), perf_mode=DR,
                        )
                    if out_bias_rows:
                        nc.tensor.matmul(
                            pso, ones_row[:, :128],
                            bo_row[:, nh * HLF:(nh + 1) * HLF],
                            start=False, stop=True, skip_group_check=True,
                        )
                    nc.vector.tensor_tensor(
                        x_list[m][:, nh * HLF:(nh + 1) * HLF], pso,
                        h_list[m][:, nh * HLF:(nh + 1) * HLF], ALU.add)

                def emit_ln1_tr(m):
                    _bld_ln(nc, pools, x_list[m][:], hb_list[m][:], gb1)
                    _bld_transpose_m(nc, pools, hTb, hb_list[m][:], m, ident[:])

                # independent PE work interleaved between scores and ctx of
                # each chain so the PE queue never blocks on an exp round-trip
                filler = {
                    (0, 0): lambda: emit_v_group(0, 1),
                    (0, 1): lambda: emit_v_group(1, 1),
                    (0, 2): lambda: emit_v_group(2, 0),
                    (0, 3): lambda: emit_v_group(3, 0),
                    (0, 4): lambda: emit_v_group(2, 1),
                    (0, 5): lambda: emit_v_group(3, 1),
                    (1, 0): lambda: emit_wo_group(0, 0),
                    (1, 1): lambda: emit_wo_group(0, 1),
                    (1, 2): lambda: (emit_wo_group(1, 0), emit_ln1_tr(0)),
                    (1, 3): lambda: emit_wo_group(1, 1),
                    (1, 5): lambda: emit_ln1_tr(1),
                }
                for s in range(NSEQ):
                    for ktile in range(KT):
                        p_half = emit_scores(s, ktile)
                        f = filler.get((s, ktile))
                        if f is not None:
                            f()
                        emit_ctx(s, ktile, p_half)

                # ---- remaining output projection + residual + LN1 ----
                for m in (2, 3):
                    emit_wo_group(m, 0)
                    emit_wo_group(m, 1)
                    emit_ln1_tr(m)

                # ---- FFN1 (transposed out + gelu) ----
                ffT = pools["ff"].tile([128, FT, TOK], F8, tag="ffT")
                for n in range(FT):
                    ps = pools["ps_proj"].tile([128, TOK], F32, tag="ps_proj")
                    for k2 in range(KT2):
                        nc.tensor.matmul(
                            ps[:], w1[:, 2 * k2:2 * k2 + 2, n * 128:(n + 1) * 128],
                            hTb[:, 2 * k2:2 * k2 + 2, :],
                            start=(k2 == 0), stop=(k2 == KT2 - 1), perf_mode=DR,
                        )
                    nc.scalar.activation(ffT[:, n, :], ps[:], AF.Gelu,
                                         bias=b1[:, n:n + 1])

                # ---- FFN2 (natural out) + residual + LN2 -> new h ----
                x2_list = [pools["h"].tile([128, H], F32, tag="h", name=f"x2_{l}_{m}")
                           for m in range(MT)]
                h_list = [pools["h"].tile([128, H], BF16, tag="h", name=f"h_{l}_{m}")
                          for m in range(MT)]
                for m in range(MT):
                    for nh in range(2):
                        ps = pools["ps_proj"].tile([128, TOK], F32, tag="ps_proj")
                        psf = ps[:, :HLF]
                        for k2 in range(FT2):
                            nc.tensor.matmul(
                                psf, ffT[:, 2 * k2:2 * k2 + 2, m * 128:(m + 1) * 128],
                                w2[:, 2 * k2:2 * k2 + 2, nh * HLF:(nh + 1) * HLF],
                                start=(k2 == 0), stop=(k2 == FT2 - 1), perf_mode=DR,
                            )
                        if out_bias_rows:
                            nc.tensor.matmul(
                                psf, ones_row[:, :128],
                                b2_row[:, nh * HLF:(nh + 1) * HLF],
                                start=False, stop=True, skip_group_check=True,
                            )
                        nc.vector.tensor_tensor(
                            x2_list[m][:, nh * HLF:(nh + 1) * HLF], psf,
                            hb_list[m][:, nh * HLF:(nh + 1) * HLF], ALU.add)
                    _bld_ln(nc, pools, x2_list[m][:], h_list[m][:], gb2)

                if debug_h:
                    for m in range(MT):
                        nc.sync.dma_start(
                            dbg[l][m * 128:(m + 1) * 128, :], h_list[m][:])

            # ---- classifier ----
            hTf = _bld_transpose(nc, pools, h_list, ident[:])
            wc = pools["bias"].tile([128, KT, T], F8, tag="wc")
            nc.sync.dma_start(wc[:], d["clf_W"].rearrange("(o p) n -> p o n", p=128))
            bc = pools["bias"].tile([T, 1], F32, tag="bc")
            nc.sync.dma_start(bc[:], d["clf_b"][:, None])
            ps = pools["ps_proj"].tile([128, TOK], F32, tag="ps_proj")
            psl = ps[:T, :]
            for k in range(KT):
                nc.tensor.matmul(psl, wc[:, k, :], hTf[:, k, :],
                                 start=(k == 0), stop=(k == KT - 1))
            lg = pools["const"].tile([T, TOK], F32, tag="lg")
            nc.scalar.activation(lg[:], psl, AF.Identity, bias=bc[:])
            nc.sync.dma_start(logitsT[:], lg[:])

    nc.compile()
    return nc


# ---------------------------------------------------------------------------
# Host side
# ---------------------------------------------------------------------------

def _np(x):
    return np.asarray(x)


def _host_embed(x, word_emb, pos_emb, type_emb, g, b):
    h = word_emb[x] + pos_emb[None, :, :] + type_emb[0][None, None, :]
    m = h.mean(-1, keepdims=True, dtype=np.float32)
    v = ((h - m) ** 2).mean(-1, keepdims=True, dtype=np.float32)
    return ((h - m) / np.sqrt(v + LN_EPS) * g + b).astype(np.float32)


def _logsumexp(a, axis):
    mx = np.max(a, axis=axis, keepdims=True)
    return (mx + np.log(np.sum(np.exp(a - mx), axis=axis, keepdims=True))).squeeze(axis)


def _host_crf(logits, target, crf_start, crf_trans, crf_end):
    logits = logits.astype(np.float32)
    mask = target > -1
    tags = np.where(mask, target, 0)
    bidx = np.arange(B)
    emit = np.take_along_axis(logits, tags[..., None], axis=-1)[..., 0]

    num = crf_start[tags[:, 0]] + emit[:, 0]
    trans = crf_trans[tags[:, :-1], tags[:, 1:]]
    num = num + np.sum((trans + emit[:, 1:]) * mask[:, 1:], axis=1)
    last = np.sum(mask.astype(np.int64), axis=1) - 1
    num = num + crf_end[tags[bidx, last]]

    alpha = crf_start[None, :] + logits[:, 0]
    for t in range(1, S):
        nxt = _logsumexp(alpha[:, :, None] + crf_trans[None], axis=1) + logits[:, t]
        alpha = np.where(mask[:, t][:, None], nxt, alpha)
    denom = _logsumexp(alpha + crf_end[None, :], axis=1)
    llh = num - denom
    return np.float32(-(llh.mean()))


def _ensure_ntff_hook():
    """Dev-only: register the axon NTFF profiling hook if the image's
    antenv package lacks axon_hooks (the boot degrades silently then)."""
    try:
        from antenv.axon_hooks import get_axon_ntff_profile_hook  # noqa: F401
        return
    except ImportError:
        pass
    try:
        import types
        import antenv
        if "/root/.axon_site" not in sys.path:
            sys.path.insert(0, "/root/.axon_site")
        from trn_agent_boot.trn_boot import _ntff_profile_via_ctypes
        hook = _ntff_profile_via_ctypes("/opt/axon/libaxon_pjrt.so")
        mod = types.ModuleType("antenv.axon_hooks")
        state = {"hook": hook}
        mod.get_axon_ntff_profile_hook = lambda: state["hook"]
        mod.set_axon_ntff_profile_hook = lambda h: state.update(hook=h)
        sys.modules["antenv.axon_hooks"] = mod
        antenv.axon_hooks = mod
    except Exception as e:  # profiling is best-effort
        print(f"[kernel] NTFF hook registration failed: {e}")


_CACHE = {}


def _get_nc(ln_affine, out_bias_rows):
    key = ("nc", ln_affine, out_bias_rows)
    if key not in _CACHE:
        _CACHE[key] = build_bert(n_layers=L, ln_affine=ln_affine,
                                 out_bias_rows=out_bias_rows)
    return _CACHE[key]


def expected_input_names(nc):
    names = set()
    for alloc in nc.m.functions[0].allocations:
        if isinstance(alloc, mybir.MemoryLocationSet) and alloc.kind == "ExternalInput":
            names.add(alloc.memorylocations[0].name)
    return names


def _prep_weights(inputs):
    bf = ml_dtypes.bfloat16
    f8 = ml_dtypes.float8_e4m3  # TRN fp8e4: inf at S.1111.000, max normal 240
    w = {}
    w["Wq"] = _np(inputs["Wq"]).astype(f8)
    w["Wk"] = _np(inputs["Wk"]).astype(f8)
    w["Wv"] = _np(inputs["Wv"]).astype(f8)
    w["Wo"] = _np(inputs["Wo"]).astype(f8)
    w["W1"] = _np(inputs["W1"]).astype(f8)
    w["W2"] = _np(inputs["W2"]).astype(f8)
    w["bq"] = _np(inputs["bq"]).astype(np.float32)
    w["bk"] = _np(inputs["bk"]).astype(np.float32)
    bo = _np(inputs["bo"]).astype(np.float32)
    bv = _np(inputs["bv"]).astype(np.float32)
    Wo = _np(inputs["Wo"]).astype(np.float32)
    # (ctx + bv) @ Wo + bo == ctx @ Wo + (bo + bv @ Wo)
    w["bo_eff"] = (bo + np.einsum("lk,lkn->ln", bv, Wo)).astype(np.float32)
    w["b1"] = _np(inputs["b1"]).astype(np.float32)
    w["b2"] = _np(inputs["b2"]).astype(np.float32)
    for nm in ("ln1_g", "ln1_b", "ln2_g", "ln2_b"):
        w[nm] = _np(inputs[nm]).astype(np.float32)
        w[nm + "_bf"] = _np(inputs[nm]).astype(bf)
    w["clf_W"] = _np(inputs["clf_W"]).astype(f8)
    w["clf_b"] = _np(inputs["clf_b"]).astype(np.float32)
    return w


def kernel(**inputs):
    x = _np(inputs["x"]).astype(np.int64)
    target = _np(inputs["target"]).astype(np.int64)
    h0 = _host_embed(
        x,
        _np(inputs["word_emb"]).astype(np.float32),
        _np(inputs["pos_emb"]).astype(np.float32),
        _np(inputs["type_emb"]).astype(np.float32),
        _np(inputs["emb_ln_g"]).astype(np.float32),
        _np(inputs["emb_ln_b"]).astype(np.float32),
    )  # [B, S, H]

    w = _prep_weights(inputs)
    ln_trivial = (
        np.all(w["ln1_g"] == 1) and np.all(w["ln2_g"] == 1)
        and np.all(w["ln1_b"] == 0) and np.all(w["ln2_b"] == 0)
    )
    ob_trivial = bool(np.all(w["bo_eff"] == 0) and np.all(w["b2"] == 0))

    nc = _get_nc(ln_affine=not ln_trivial, out_bias_rows=not ob_trivial)
    expected = expected_input_names(nc)
    in_maps = []
    for c in range(N_CORES):
        im = {k: v for k, v in w.items() if k in expected}
        im["h0"] = np.ascontiguousarray(
            h0[c * NSEQ:(c + 1) * NSEQ].reshape(TOK, H)).astype(ml_dtypes.bfloat16)
        in_maps.append(im)

    import os
    trace_dir = os.environ.get("BERT_KERNEL_TRACE", "")
    kwargs = {}
    if trace_dir:
        _ensure_ntff_hook()
        os.makedirs(trace_dir, exist_ok=True)
        kwargs = dict(trace=True, tmpdir=trace_dir)
    res = None
    last_err = None
    for attempt in range(3):
        try:
            res = bass_utils.run_bass_kernel_spmd(
                nc, in_maps, core_ids=list(range(N_CORES)), **kwargs)
            break
        except Exception as e:  # transient device errors (NRT_EXEC_UNIT_...)
            last_err = e
            import time as _time
            _time.sleep(5)
    if res is None:
        raise last_err
    if trace_dir:
        print(f"[kernel] exec_time_ns: {res.exec_time_ns}")
        _CACHE["last_results"] = res
    logits = np.empty((B, S, T), np.float32)
    for c in range(N_CORES):
        lt = res.results[c]["logitsT"]  # [T, TOK]
        logits[c * NSEQ:(c + 1) * NSEQ] = lt.T.reshape(NSEQ, S, T)

    return _host_crf(
        logits, target,
        _np(inputs["crf_start"]).astype(np.float32),
        _np(inputs["crf_trans"]).astype(np.float32),
        _np(inputs["crf_end"]).astype(np.float32),
    )



# revision 36
# speedup vs baseline: 1.5128x; 1.0394x over previous
"""BERT-base + CRF loss kernel for 8x Trainium2 NeuronCores.

Strategy (hardcoded for B=16, S=256, H=768, NH=12, DFF=3072, L=12, T=9):
  - Data-parallel over batch: core c processes sequences (2c, 2c+1).
  - Host: embedding gather + embedding LayerNorm (tiny), CRF forward
    algorithm on the [B,S,9] emissions (0.00005% of FLOPs, inherently
    sequential), and folding of zero-cost algebra (bv folded into an
    effective output-projection bias).
  - Device (per core): 12 transformer encoder layers + classifier head
    on 512 tokens. Matmuls in bf16 with fp32 PSUM accumulation;
    residual stream + layernorm statistics in fp32.

Layout: activations are kept in natural layout h[tokens, feat] (so
LayerNorm reduces along the free dim) and transposed per-sublayer into
hT[feat, tokens] via PE-transposes to serve as matmul operands.
Attention computes scoresT[j,i] = k.q per (seq, head), exp on ACT, and
the softmax denominator falls out of the ctx matmul via a ones-column
appended to V. The division by the denominator (free-dim broadcast) is
done with a gpsimd partition_broadcast + one DVE multiply that doubles
as the PSUM->SBUF copy.
"""

import sys

if "/opt/trn_rl_repo" not in sys.path:
    sys.path.insert(0, "/opt/trn_rl_repo")

import contextlib

import numpy as np
import ml_dtypes

import concourse.bass as bass
import concourse.tile as tile
from concourse import bacc, mybir
from concourse import bass_utils
from concourse.masks import make_identity

F32 = mybir.dt.float32
BF16 = mybir.dt.bfloat16
F8 = mybir.dt.float8e4
AF = mybir.ActivationFunctionType
ALU = mybir.AluOpType
DR = mybir.MatmulPerfMode.DoubleRow

B, S, V, H, NH, DFF, L, T = 16, 256, 30522, 768, 12, 3072, 12, 9
DH = H // NH  # 64
LN_EPS = 1e-12
N_CORES = 8
TOK = (B // N_CORES) * S  # 512 tokens per core
KT = H // 128  # 6 k-tiles
KT2 = KT // 2  # 3 DoubleRow k-pairs
MT = TOK // 128  # 4 m-tiles
FT = DFF // 128  # 24 ff tiles
FT2 = FT // 2  # 12 DoubleRow ff-pairs
NSEQ = B // N_CORES  # 2 sequences per core
HLF = H // 2  # 384, n-half for natural-layout outputs
VW = DH + 1  # 65: v columns per head incl. ones column
VPAD = 784  # NH*VW (=780) padded so the v m-stride is 16B-aligned for DoubleRow


def _bld_ln_stats(nc, pools, x, half, stats=None):
    """Emit bn_stats for one nh-half of x right after its residual add."""
    if stats is None:
        stats = pools["small"].tile([128, 2, 6], F32, tag="ln_stats")
    nc.vector.bn_stats(stats[:, half, :], x[:, half * HLF:(half + 1) * HLF])
    return stats


def _bld_ln(nc, pools, x, h, gb_sb=None, eps=LN_EPS, stats=None):
    """LayerNorm along free dim for one m-tile. x: [128, H] f32, h bf16.

    If `stats` (from _bld_ln_stats on both halves) is given, the stats
    passes are already done and only the finish chain is emitted."""
    small = pools["small"]
    if stats is None:
        stats = _bld_ln_stats(nc, pools, x, 0)
        _bld_ln_stats(nc, pools, x, 1, stats)
    mv = small.tile([128, 2], F32, tag="ln_mv")
    nc.vector.bn_aggr(mv[:], stats[:])
    std = small.tile([128, 1], F32, tag="ln_std")
    nc.scalar.activation(std[:], mv[:, 1:2], AF.Sqrt, bias=float(eps))
    rstd = small.tile([128, 1], F32, tag="ln_rstd")
    nc.vector.reciprocal_approx_fast(rstd[:], std[:])
    # apply in halves: consumers of the low half (transposes k<3) unblock
    # one DVE-pass earlier
    for hf in range(2):
        sl = slice(hf * HLF, (hf + 1) * HLF)
        nc.vector.tensor_scalar(h[:, sl], x[:, sl], mv[:, 0:1], rstd[:],
                                ALU.subtract, ALU.mult)
    if gb_sb is not None:
        g_b, b_b = gb_sb
        nc.vector.tensor_tensor(h[:], h[:], g_b[:], ALU.mult)
        nc.vector.tensor_tensor(h[:], h[:], b_b[:], ALU.add)


def _bld_transpose_m(nc, pools, hT, h_m, m, ident):
    """One m-tile [128, H] bf16 natural -> hT[:, :, m-block] fp8.

    All KT 128x128 transposes pack into one PSUM bank (bf16: 6*128 cols
    = 1.5KB) so a single strided copy drains it."""
    pt = pools["ps_attn"].tile([128, KT * 128], BF16, tag="ps_attn")
    for k in range(KT):
        nc.tensor.matmul(
            pt[:, k * 128:(k + 1) * 128],
            h_m[:, k * 128:(k + 1) * 128], ident,
            is_transpose=True, start=(k == 0), stop=(k == KT - 1),
            skip_group_check=True,
        )
    nc.any.tensor_copy(
        hT[:, :, m * 128:(m + 1) * 128],
        pt.rearrange("p (k c) -> p k c", c=128))


def _bld_transpose(nc, pools, h_list, ident):
    """h_list: MT tiles [128, H] bf16 natural -> hT [128, KT, TOK] fp8."""
    hT = pools["hT"].tile([128, KT, TOK], F8, tag="hT")
    for m in range(MT):
        _bld_transpose_m(nc, pools, hT, h_list[m][:], m, ident)
    return hT


def _bld_proj_T(nc, pools, w_sb, hT, bias_col, out_tag):
    """Transposed-output projection: per-k output tiles [128, TOK] bf16.

    DoubleRow fp8: each matmul consumes a pair of 128-row k-tiles."""
    outs = []
    for n in range(KT):
        out = pools["flow"].tile([128, TOK], BF16, tag=out_tag)
        ps = pools["ps_proj"].tile([128, TOK], F32, tag="ps_proj")
        for k2 in range(KT2):
            nc.tensor.matmul(
                ps[:], w_sb[:, 2 * k2:2 * k2 + 2, n * 128:(n + 1) * 128],
                hT[:, 2 * k2:2 * k2 + 2, :],
                start=(k2 == 0), stop=(k2 == KT2 - 1), perf_mode=DR,
            )
        if bias_col is not None:
            # bias-add on DVE: keeps ACT free for exp/gelu (fewer table loads)
            nc.vector.tensor_scalar(out[:], ps[:], bias_col[:, n:n + 1], None,
                                    ALU.add)
        else:
            nc.scalar.copy(out[:], ps[:])
        outs.append(out)
    return outs


def build_bert(n_layers=L, ln_affine=False, out_bias_rows=False, debug_h=False):
    """Build the bass program. Returns nc.

    ln_affine: emit gamma/beta application (needed when ln params are
    not identity). out_bias_rows: emit ones-row matmuls adding bo_eff/b2
    (needed when those are nonzero)."""
    nc = bacc.Bacc("TRN2", target_bir_lowering=False, debug=False,
                   enable_asserts=False, num_devices=N_CORES)

    d = {}
    d["h0"] = nc.dram_tensor("h0", [TOK, H], BF16, kind="ExternalInput").ap()
    for nm in ("Wq", "Wk", "Wv", "Wo"):
        d[nm] = nc.dram_tensor(nm, [n_layers, H, H], F8, kind="ExternalInput").ap()
    d["W1"] = nc.dram_tensor("W1", [n_layers, H, DFF], F8, kind="ExternalInput").ap()
    d["W2"] = nc.dram_tensor("W2", [n_layers, DFF, H], F8, kind="ExternalInput").ap()
    for nm in ("bq", "bk"):
        d[nm] = nc.dram_tensor(nm, [n_layers, H], F32, kind="ExternalInput").ap()
    if out_bias_rows:
        for nm in ("bo_eff", "b2"):
            d[nm] = nc.dram_tensor(nm, [n_layers, H], F32, kind="ExternalInput").ap()
    if ln_affine:
        for nm in ("ln1_g", "ln1_b", "ln2_g", "ln2_b"):
            d[nm + "_bf"] = nc.dram_tensor(nm + "_bf", [n_layers, H], BF16,
                                           kind="ExternalInput").ap()
    d["b1"] = nc.dram_tensor("b1", [n_layers, DFF], F32, kind="ExternalInput").ap()
    d["clf_W"] = nc.dram_tensor("clf_W", [H, T], F8, kind="ExternalInput").ap()
    d["clf_b"] = nc.dram_tensor("clf_b", [T], F32, kind="ExternalInput").ap()
    logitsT = nc.dram_tensor("logitsT", [T, TOK], F32, kind="ExternalOutput").ap()
    if debug_h:
        dbg = nc.dram_tensor("dbg_h", [n_layers, TOK, H], F32,
                             kind="ExternalOutput").ap()

    with tile.TileContext(nc) as tc:
        with contextlib.ExitStack() as ctx:
            pools = {
                # f32 residual-stream m-tiles [128, H] (3KB/partition each)
                "h": ctx.enter_context(tc.tile_pool(name="h", bufs=8)),
                # per-k bf16 flow tiles [128, TOK] (qT/kT)
                "flow": ctx.enter_context(tc.tile_pool(name="flow", bufs=KT)),
                "v": ctx.enter_context(tc.tile_pool(name="v", bufs=2)),
                "hT": ctx.enter_context(tc.tile_pool(name="hT", bufs=2)),
                "p": ctx.enter_context(tc.tile_pool(name="p", bufs=6)),
                "ff": ctx.enter_context(tc.tile_pool(name="ff", bufs=2)),
                "w": ctx.enter_context(tc.tile_pool(name="w", bufs=6)),
                "lnb": ctx.enter_context(tc.tile_pool(name="lnb", bufs=1)),
                "wff": ctx.enter_context(tc.tile_pool(name="wff", bufs=2)),
                "bias": ctx.enter_context(
                    tc.tile_pool(name="bias", bufs=1 if ln_affine else 2)),
                "small": ctx.enter_context(tc.tile_pool(name="small", bufs=4)),
                "small2": ctx.enter_context(tc.tile_pool(name="small2", bufs=2)),
                "const": ctx.enter_context(tc.tile_pool(name="const", bufs=1)),
                "ps_proj": ctx.enter_context(
                    tc.tile_pool(name="ps_proj", bufs=3, space="PSUM")),
                "ps_attn": ctx.enter_context(
                    tc.tile_pool(name="ps_attn", bufs=3, space="PSUM")),
                "ps_ctx": ctx.enter_context(
                    tc.tile_pool(name="ps_ctx", bufs=2, space="PSUM")),
            }

            ident = pools["const"].tile([128, 128], BF16, tag="ident")
            make_identity(nc, ident[:])
            # scratch for dummy activations that pre-load ACT tables during
            # idle windows (a table swap is ~1.3us and otherwise lands on
            # the first exp of each layer's attention, stalling its ctx)
            warm = pools["const"].tile([1, 1], F32, tag="warm")
            nc.vector.memset(warm[:], 0.0)
            # const APs used by nc.scalar.activation float-bias conversion
            zero_c = pools["const"].tile([128, 1], F32, tag="zero_c")
            nc.vector.memset(zero_c[:], 0.0)
            nc.const_aps.aps[(F32, 0.0)] = zero_c[:]
            eps_c = pools["const"].tile([128, 1], F32, tag="eps_c")
            nc.vector.memset(eps_c[:], float(LN_EPS))
            nc.const_aps.aps[(F32, float(LN_EPS))] = eps_c[:]
            if out_bias_rows:
                ones_row = pools["const"].tile([1, 128], F32, tag="ones_row")
                nc.vector.memset(ones_row[:], 1.0)

            h_list = []
            for m in range(MT):
                hm = pools["h"].tile([128, H], BF16, tag="h")
                nc.sync.dma_start(
                    hm[:], d["h0"][m * 128:(m + 1) * 128, :])
                h_list.append(hm)
            # rolling feature-major transpose of the residual stream: built
            # here for layer 0, then per-m inside each layer's LN2 tail
            hT = _bld_transpose(nc, pools, h_list, ident[:])

            for l in range(n_layers):
                # pre-load the Exp table while ACT is idle (QKV window)
                nc.scalar.activation(warm[:], warm[:], AF.Exp)
                # ---- per-layer weights/bias loads ----
                def _load_wproj(nm):
                    wt = pools["w"].tile([128, KT, H], F8, tag="wproj",
                                         name=f"{nm}_{l}")
                    src_ap = d[nm][l].rearrange("(o p) n -> p o n", p=128)
                    # per-k-pair chunks: finer deps let the first matmuls of
                    # each accumulation start before the whole tensor lands
                    for kk in range(KT2):
                        nc.sync.dma_start(wt[:, 2 * kk:2 * kk + 2, :],
                                          src_ap[:, 2 * kk:2 * kk + 2, :])
                    return wt
                wq = _load_wproj("Wq")
                wk = _load_wproj("Wk")
                wv = _load_wproj("Wv")
                wo = _load_wproj("Wo")
                w1 = pools["wff"].tile([128, KT, DFF], F8, tag="w1")
                nc.sync.dma_start(w1[:], d["W1"][l].rearrange("(o p) n -> p o n", p=128))
                w2 = pools["wff"].tile([128, FT, H], F8, tag="w2")
                nc.sync.dma_start(w2[:], d["W2"][l].rearrange("(o p) n -> p o n", p=128))

                bq = pools["bias"].tile([128, KT], F32, tag="bq")
                nc.sync.dma_start(bq[:], d["bq"][l].rearrange("(o p) -> p o", p=128))
                bk = pools["bias"].tile([128, KT], F32, tag="bk")
                nc.sync.dma_start(bk[:], d["bk"][l].rearrange("(o p) -> p o", p=128))
                b1 = pools["bias"].tile([128, FT], F32, tag="b1")
                nc.sync.dma_start(b1[:], d["b1"][l].rearrange("(o p) -> p o", p=128))

                gb1 = gb2 = None
                if ln_affine:
                    def _ln_bcast(nm):
                        bcast = pools["lnb"].tile([128, H], BF16, tag=nm + "_b")
                        nc.sync.dma_start(
                            bcast[:], d[nm + "_bf"][l][None, :].partition_broadcast(128))
                        return bcast
                    gb1 = [_ln_bcast("ln1_g"), _ln_bcast("ln1_b")]
                    gb2 = [_ln_bcast("ln2_g"), _ln_bcast("ln2_b")]
                bo_row = b2_row = None
                if out_bias_rows:
                    bo_row = pools["lnb"].tile([1, H], F32, tag="bo_row")
                    nc.sync.dma_start(bo_row[:], d["bo_eff"][l][None, :])
                    b2_row = pools["lnb"].tile([1, H], F32, tag="b2_row")
                    nc.sync.dma_start(b2_row[:], d["b2"][l][None, :])

                # ---- Q, K projections (transposed out, per-k tiles) ----
                # (hT was produced by the previous layer's LN2 tail)
                qT = _bld_proj_T(nc, pools, wq, hT, bq, "qT")
                kT = _bld_proj_T(nc, pools, wk, hT, bk, "kT")

                # ---- V projection (natural out, no bias) + ones cols ----
                # single fp8 tile [128, MT, VPAD]: m-pairs are DoubleRow
                # contraction pairs for the ctx matmul. Only the (m, nh=0)
                # groups for seq0's m-tiles are emitted up front; the rest
                # interleave into seq0's softmax-chain window to keep PE fed.
                v_sb = pools["v"].tile([128, MT, VPAD], F8, tag="v")
                nc.vector.memset(
                    v_sb[:, :, :NH * VW].rearrange(
                        "p m (h w) -> p m h w", w=VW)[:, :, :, DH], 1.0)

                def emit_v_group(m, nh):
                    ps = pools["ps_proj"].tile([128, TOK], F32, tag="ps_proj")
                    psv = ps[:, :HLF]
                    for k2 in range(KT2):
                        nc.tensor.matmul(
                            psv, hT[:, 2 * k2:2 * k2 + 2, m * 128:(m + 1) * 128],
                            wv[:, 2 * k2:2 * k2 + 2, nh * HLF:(nh + 1) * HLF],
                            start=(k2 == 0), stop=(k2 == KT2 - 1), perf_mode=DR,
                        )
                    # one strided copy drains all 6 heads of this half
                    nh0 = nh * (NH // 2)
                    nc.vector.tensor_copy(
                        v_sb[:, m, nh0 * VW:nh0 * VW + 6 * VW].rearrange(
                            "p (h w) -> p h w", w=VW)[:, :, :DH],
                        psv.rearrange("p (h w) -> p h w", w=DH))

                emit_v_group(0, 0)
                emit_v_group(1, 0)

                # ---- attention: seq-outer. Chains for seq s run their
                # softmax round-trips (ACT exp -> PE ctx -> DVE/gpsimd
                # normalize) while PE interleaves independent work: remaining
                # V groups during seq0, seq0's output projection + LN1 +
                # transposes during seq1. Per (ktile, seq) both head-halves
                # share one scores bank, one ctx bank, one recip/broadcast. --
                ctxT = {
                    si: pools["hT"].tile([128, KT, 256], F8, tag="ctxT",
                                         name=f"ctx_{l}_{si}")
                    for si in range(NSEQ)
                }
                x_list = [pools["h"].tile([128, H], F32, tag="h", name=f"x_{l}_{m}")
                          for m in range(MT)]
                hb_list = [pools["h"].tile([128, H], BF16, tag="h", name=f"hb_{l}_{m}")
                           for m in range(MT)]
                hTb = pools["hT"].tile([128, KT, TOK], F8, tag="hT",
                                       name=f"hTb_{l}")

                def emit_scores(s, ktile):
                    p_half = []
                    for half in range(2):
                        r0 = half * 64
                        # both jt score blocks into one PSUM bank so a
                        # single exp covers them (start=True clears the
                        # bank; the second matmul overwrites its region)
                        ps_s = pools["ps_attn"].tile([128, 512], F32,
                                                     tag="ps_attn")
                        for jt in range(2):
                            nc.tensor.matmul(
                                ps_s[:, jt * 256:(jt + 1) * 256],
                                kT[ktile][r0:r0 + 64,
                                   s * 256 + jt * 128:s * 256 + (jt + 1) * 128],
                                qT[ktile][r0:r0 + 64, s * 256:(s + 1) * 256],
                                start=(jt == 0), stop=(jt == 1),
                                skip_group_check=True,
                            )
                        # p = exp(score / sqrt(dh)); no max-subtraction
                        # needed: |score/8| is O(1) here and exp is fp32.
                        p_sb = pools["p"].tile([128, 2, 256], F8, tag="p",
                                               name=f"p_{l}_{ktile}_{s}_{half}")
                        nc.scalar.activation(
                            p_sb.rearrange("p a b -> p (a b)"), ps_s[:],
                            AF.Exp, scale=float(1.0 / np.sqrt(DH)))
                        p_half.append(p_sb)
                    return p_half

                def emit_ctx(s, ktile, p_half):
                    # both halves' ctx into one bank: cols 0:256 half0,
                    # 256:512 half1 -> one sums-row copy/recip/broadcast
                    ps_c = pools["ps_ctx"].tile([VW, 512], F32, tag="ps_ctx")
                    for half in range(2):
                        hd = 2 * ktile + half
                        nc.tensor.matmul(
                            ps_c[:, half * 256:(half + 1) * 256],
                            v_sb[:, 2 * s:2 * s + 2, hd * VW:hd * VW + VW],
                            p_half[half][:],
                            start=(half == 0), stop=(half == 1),
                            skip_group_check=True, perf_mode=DR,
                        )
                    # custom-DVE ops misread PSUM operands on HW: bounce the
                    # sums row through SBUF (on ACT; DVE is busier) first.
                    r_sb = pools["small2"].tile([1, 512], F32, tag="r")
                    nc.scalar.copy(r_sb[:], ps_c[DH:VW, :])
                    nc.vector.reciprocal_approx_fast(r_sb[:], r_sb[:])
                    rb = pools["small2"].tile([64, 512], F32, tag="rb")
                    nc.gpsimd.partition_broadcast(rb[:], r_sb[:])
                    for half in range(2):
                        r0 = half * 64
                        nc.vector.tensor_tensor(
                            ctxT[s][r0:r0 + 64, ktile, :],
                            ps_c[:DH, half * 256:(half + 1) * 256],
                            rb[:, half * 256:(half + 1) * 256], ALU.mult)

                def emit_wo_group(m, nh):
                    ps = pools["ps_proj"].tile([128, TOK], F32, tag="ps_proj")
                    pso = ps[:, :HLF]
                    for k2 in range(KT2):
                        nc.tensor.matmul(
                            pso, ctxT[m // 2][:, 2 * k2:2 * k2 + 2,
                                              (m % 2) * 128:(m % 2 + 1) * 128],
                            wo[:, 2 * k2:2 * k2 + 2, nh * HLF:(nh + 1) * HLF],
                            start=(k2 == 0), stop=(k2 == KT2 - 1), perf_mode=DR,
                        )
                    if out_bias_rows:
                        nc.tensor.matmul(
                            pso, ones_row[:, :128],
                            bo_row[:, nh * HLF:(nh + 1) * HLF],
                            start=False, stop=True, skip_group_check=True,
                        )
                    nc.vector.tensor_tensor(
                        x_list[m][:, nh * HLF:(nh + 1) * HLF], pso,
                        h_list[m][:, nh * HLF:(nh + 1) * HLF], ALU.add)
                    # half-stats right after the add: shortens the LN finish
                    ln1_stats[m] = _bld_ln_stats(nc, pools, x_list[m][:], nh,
                                                 ln1_stats.get(m))

                ln1_stats = {}

                def emit_ln1(m):
                    _bld_ln(nc, pools, x_list[m][:], hb_list[m][:], gb1,
                            stats=ln1_stats[m])

                def emit_tr1(m):
                    _bld_transpose_m(nc, pools, hTb, hb_list[m][:], m, ident[:])

                # independent PE work interleaved between scores and ctx of
                # each chain so the PE queue never blocks on an exp round-trip;
                # each transpose trails its LN finish by ~2 chain periods
                filler = {
                    (0, 0): lambda: emit_v_group(0, 1),
                    (0, 1): lambda: emit_v_group(1, 1),
                    (0, 2): lambda: emit_v_group(2, 0),
                    (0, 3): lambda: emit_v_group(3, 0),
                    (0, 4): lambda: emit_v_group(2, 1),
                    (0, 5): lambda: emit_v_group(3, 1),
                    (1, 0): lambda: emit_wo_group(0, 0),
                    (1, 1): lambda: (emit_wo_group(0, 1), emit_ln1(0)),
                    (1, 2): lambda: emit_wo_group(1, 0),
                    (1, 3): lambda: (emit_wo_group(1, 1), emit_ln1(1)),
                }
                for s in range(NSEQ):
                    for ktile in range(KT):
                        p_half = emit_scores(s, ktile)
                        f = filler.get((s, ktile))
                        if f is not None:
                            f()
                        emit_ctx(s, ktile, p_half)

                # ---- remaining output projection + residual + LN1; the
                # m0/m1 transposes land here so PE has work while the last
                # chains normalize (and their PSUM-drain copies hit DVE
                # after the chain mults are done) ----
                emit_tr1(0)
                emit_tr1(1)
                for m in (2, 3):
                    emit_wo_group(m, 0)
                    emit_wo_group(m, 1)
                    emit_ln1(m)
                emit_tr1(2)
                emit_tr1(3)

                # ---- FFN1 (transposed out + gelu) ----
                ffT = pools["ff"].tile([128, FT, TOK], F8, tag="ffT")
                for n in range(FT):
                    ps = pools["ps_proj"].tile([128, TOK], F32, tag="ps_proj")
                    for k2 in range(KT2):
                        nc.tensor.matmul(
                            ps[:], w1[:, 2 * k2:2 * k2 + 2, n * 128:(n + 1) * 128],
                            hTb[:, 2 * k2:2 * k2 + 2, :],
                            start=(k2 == 0), stop=(k2 == KT2 - 1), perf_mode=DR,
                        )
                    nc.scalar.activation(ffT[:, n, :], ps[:], AF.Gelu,
                                         bias=b1[:, n:n + 1])

                # ---- FFN2 (natural out) + residual + LN2 -> new h; the
                # next layer's hT transposes roll in per-m so the layer
                # boundary never serializes on the LN2 chains ----
                x2_list = [pools["h"].tile([128, H], F32, tag="h", name=f"x2_{l}_{m}")
                           for m in range(MT)]
                h_list = [pools["h"].tile([128, H], BF16, tag="h", name=f"h_{l}_{m}")
                          for m in range(MT)]
                hT_next = pools["hT"].tile([128, KT, TOK], F8, tag="hT",
                                           name=f"hT_{l + 1}")
                ln2_stats = {}
                for m in range(MT):
                    for nh in range(2):
                        ps = pools["ps_proj"].tile([128, TOK], F32, tag="ps_proj")
                        psf = ps[:, :HLF]
                        for k2 in range(FT2):
                            nc.tensor.matmul(
                                psf, ffT[:, 2 * k2:2 * k2 + 2, m * 128:(m + 1) * 128],
                                w2[:, 2 * k2:2 * k2 + 2, nh * HLF:(nh + 1) * HLF],
                                start=(k2 == 0), stop=(k2 == FT2 - 1), perf_mode=DR,
                            )
                        if out_bias_rows:
                            nc.tensor.matmul(
                                psf, ones_row[:, :128],
                                b2_row[:, nh * HLF:(nh + 1) * HLF],
                                start=False, stop=True, skip_group_check=True,
                            )
                        nc.vector.tensor_tensor(
                            x2_list[m][:, nh * HLF:(nh + 1) * HLF], psf,
                            hb_list[m][:, nh * HLF:(nh + 1) * HLF], ALU.add)
                        ln2_stats[m] = _bld_ln_stats(nc, pools, x2_list[m][:],
                                                     nh, ln2_stats.get(m))
                    _bld_ln(nc, pools, x2_list[m][:], h_list[m][:], gb2,
                            stats=ln2_stats[m])
                    if m > 0:
                        # transpose trails its LN by one m-tile of FFN2 work
                        _bld_transpose_m(nc, pools, hT_next, h_list[m - 1][:],
                                         m - 1, ident[:])
                _bld_transpose_m(nc, pools, hT_next, h_list[MT - 1][:],
                                 MT - 1, ident[:])
                hT = hT_next

                if debug_h:
                    for m in range(MT):
                        nc.sync.dma_start(
                            dbg[l][m * 128:(m + 1) * 128, :], h_list[m][:])

            # ---- classifier ----
            hTf = hT
            wc = pools["bias"].tile([128, KT, T], F8, tag="wc")
            nc.sync.dma_start(wc[:], d["clf_W"].rearrange("(o p) n -> p o n", p=128))
            bc = pools["bias"].tile([T, 1], F32, tag="bc")
            nc.sync.dma_start(bc[:], d["clf_b"][:, None])
            ps = pools["ps_proj"].tile([128, TOK], F32, tag="ps_proj")
            psl = ps[:T, :]
            for k in range(KT):
                nc.tensor.matmul(psl, wc[:, k, :], hTf[:, k, :],
                                 start=(k == 0), stop=(k == KT - 1))
            lg = pools["const"].tile([T, TOK], F32, tag="lg")
            nc.scalar.activation(lg[:], psl, AF.Identity, bias=bc[:])
            nc.sync.dma_start(logitsT[:], lg[:])

    nc.compile()
    return nc


# ---------------------------------------------------------------------------
# Host side
# ---------------------------------------------------------------------------

def _np(x):
    return np.asarray(x)


def _host_embed(x, word_emb, pos_emb, type_emb, g, b):
    h = word_emb[x] + pos_emb[None, :, :] + type_emb[0][None, None, :]
    m = h.mean(-1, keepdims=True, dtype=np.float32)
    v = ((h - m) ** 2).mean(-1, keepdims=True, dtype=np.float32)
    return ((h - m) / np.sqrt(v + LN_EPS) * g + b).astype(np.float32)


def _logsumexp(a, axis):
    mx = np.max(a, axis=axis, keepdims=True)
    return (mx + np.log(np.sum(np.exp(a - mx), axis=axis, keepdims=True))).squeeze(axis)


def _host_crf(logits, target, crf_start, crf_trans, crf_end):
    logits = logits.astype(np.float32)
    mask = target > -1
    tags = np.where(mask, target, 0)
    bidx = np.arange(B)
    emit = np.take_along_axis(logits, tags[..., None], axis=-1)[..., 0]

    num = crf_start[tags[:, 0]] + emit[:, 0]
    trans = crf_trans[tags[:, :-1], tags[:, 1:]]
    num = num + np.sum((trans + emit[:, 1:]) * mask[:, 1:], axis=1)
    last = np.sum(mask.astype(np.int64), axis=1) - 1
    num = num + crf_end[tags[bidx, last]]

    alpha = crf_start[None, :] + logits[:, 0]
    for t in range(1, S):
        nxt = _logsumexp(alpha[:, :, None] + crf_trans[None], axis=1) + logits[:, t]
        alpha = np.where(mask[:, t][:, None], nxt, alpha)
    denom = _logsumexp(alpha + crf_end[None, :], axis=1)
    llh = num - denom
    return np.float32(-(llh.mean()))


def _ensure_ntff_hook():
    """Dev-only: register the axon NTFF profiling hook if the image's
    antenv package lacks axon_hooks (the boot degrades silently then)."""
    try:
        from antenv.axon_hooks import get_axon_ntff_profile_hook  # noqa: F401
        return
    except ImportError:
        pass
    try:
        import types
        import antenv
        if "/root/.axon_site" not in sys.path:
            sys.path.insert(0, "/root/.axon_site")
        from trn_agent_boot.trn_boot import _ntff_profile_via_ctypes
        hook = _ntff_profile_via_ctypes("/opt/axon/libaxon_pjrt.so")
        mod = types.ModuleType("antenv.axon_hooks")
        state = {"hook": hook}
        mod.get_axon_ntff_profile_hook = lambda: state["hook"]
        mod.set_axon_ntff_profile_hook = lambda h: state.update(hook=h)
        sys.modules["antenv.axon_hooks"] = mod
        antenv.axon_hooks = mod
    except Exception as e:  # profiling is best-effort
        print(f"[kernel] NTFF hook registration failed: {e}")


_CACHE = {}


def _get_nc(ln_affine, out_bias_rows):
    key = ("nc", ln_affine, out_bias_rows)
    if key not in _CACHE:
        _CACHE[key] = build_bert(n_layers=L, ln_affine=ln_affine,
                                 out_bias_rows=out_bias_rows)
    return _CACHE[key]


def expected_input_names(nc):
    names = set()
    for alloc in nc.m.functions[0].allocations:
        if isinstance(alloc, mybir.MemoryLocationSet) and alloc.kind == "ExternalInput":
            names.add(alloc.memorylocations[0].name)
    return names


def _prep_weights(inputs):
    bf = ml_dtypes.bfloat16
    f8 = ml_dtypes.float8_e4m3  # TRN fp8e4: inf at S.1111.000, max normal 240
    w = {}
    w["Wq"] = _np(inputs["Wq"]).astype(f8)
    w["Wk"] = _np(inputs["Wk"]).astype(f8)
    w["Wv"] = _np(inputs["Wv"]).astype(f8)
    w["Wo"] = _np(inputs["Wo"]).astype(f8)
    w["W1"] = _np(inputs["W1"]).astype(f8)
    w["W2"] = _np(inputs["W2"]).astype(f8)
    w["bq"] = _np(inputs["bq"]).astype(np.float32)
    w["bk"] = _np(inputs["bk"]).astype(np.float32)
    bo = _np(inputs["bo"]).astype(np.float32)
    bv = _np(inputs["bv"]).astype(np.float32)
    Wo = _np(inputs["Wo"]).astype(np.float32)
    # (ctx + bv) @ Wo + bo == ctx @ Wo + (bo + bv @ Wo)
    w["bo_eff"] = (bo + np.einsum("lk,lkn->ln", bv, Wo)).astype(np.float32)
    w["b1"] = _np(inputs["b1"]).astype(np.float32)
    w["b2"] = _np(inputs["b2"]).astype(np.float32)
    for nm in ("ln1_g", "ln1_b", "ln2_g", "ln2_b"):
        w[nm] = _np(inputs[nm]).astype(np.float32)
        w[nm + "_bf"] = _np(inputs[nm]).astype(bf)
    w["clf_W"] = _np(inputs["clf_W"]).astype(f8)
    w["clf_b"] = _np(inputs["clf_b"]).astype(np.float32)
    return w


def kernel(**inputs):
    x = _np(inputs["x"]).astype(np.int64)
    target = _np(inputs["target"]).astype(np.int64)
    h0 = _host_embed(
        x,
        _np(inputs["word_emb"]).astype(np.float32),
        _np(inputs["pos_emb"]).astype(np.float32),
        _np(inputs["type_emb"]).astype(np.float32),
        _np(inputs["emb_ln_g"]).astype(np.float32),
        _np(inputs["emb_ln_b"]).astype(np.float32),
    )  # [B, S, H]

    w = _prep_weights(inputs)
    ln_trivial = (
        np.all(w["ln1_g"] == 1) and np.all(w["ln2_g"] == 1)
        and np.all(w["ln1_b"] == 0) and np.all(w["ln2_b"] == 0)
    )
    ob_trivial = bool(np.all(w["bo_eff"] == 0) and np.all(w["b2"] == 0))

    nc = _get_nc(ln_affine=not ln_trivial, out_bias_rows=not ob_trivial)
    expected = expected_input_names(nc)
    in_maps = []
    for c in range(N_CORES):
        im = {k: v for k, v in w.items() if k in expected}
        im["h0"] = np.ascontiguousarray(
            h0[c * NSEQ:(c + 1) * NSEQ].reshape(TOK, H)).astype(ml_dtypes.bfloat16)
        in_maps.append(im)

    import os
    trace_dir = os.environ.get("BERT_KERNEL_TRACE", "")
    kwargs = {}
    if trace_dir:
        _ensure_ntff_hook()
        os.makedirs(trace_dir, exist_ok=True)
        kwargs = dict(trace=True, tmpdir=trace_dir)
    res = None
    last_err = None
    for attempt in range(3):
        try:
            res = bass_utils.run_bass_kernel_spmd(
                nc, in_maps, core_ids=list(range(N_CORES)), **kwargs)
            break
        except Exception as e:  # transient device errors (NRT_EXEC_UNIT_...)
            last_err = e
            import time as _time
            _time.sleep(5)
    if res is None:
        raise last_err
    if trace_dir:
        print(f"[kernel] exec_time_ns: {res.exec_time_ns}")
        _CACHE["last_results"] = res
    logits = np.empty((B, S, T), np.float32)
    for c in range(N_CORES):
        lt = res.results[c]["logitsT"]  # [T, TOK]
        logits[c * NSEQ:(c + 1) * NSEQ] = lt.T.reshape(NSEQ, S, T)

    return _host_crf(
        logits, target,
        _np(inputs["crf_start"]).astype(np.float32),
        _np(inputs["crf_trans"]).astype(np.float32),
        _np(inputs["crf_end"]).astype(np.float32),
    )

